# revision 6
# baseline (speedup 1.0000x reference)
"""Causal self-attention (dense transformer block) for 8 Trainium2 NeuronCores.

Sharding: DP over batch (2) x TP over heads (4 groups of 4 heads) = 8 cores.
Per core: column-parallel QKV projection (4 heads), RoPE, causal
flash-attention (no-max-subtraction softmax with constant bias), row-parallel
output projection producing a partial [oc, t] result; host sums the 4 TP
partials per batch and transposes back.

Device matmuls run as float32r (full-rate fp32 storage, ~tf32 accuracy).

Self-contained: hardcodes shapes, builds/compiles/runs the Bass kernel via
run_bass_kernel_spmd on cores 0-7.
"""

import os
import sys
import types

sys.path.insert(0, "/opt/trn_rl_repo")

import numpy as np

import concourse.bass as bass
import concourse.mybir as mybir
import concourse.tile as tile
from concourse import bacc
from concourse.bass_utils import run_bass_kernel_spmd
from concourse.vector_clock import ScopedClock, VectorClock

F32 = mybir.dt.float32
F32R = mybir.dt.float32r
AF = mybir.ActivationFunctionType
ALU = mybir.AluOpType

P = 128
T = 2048
C = 2048
NH = 16          # total heads
HPC = 4          # heads per core
HSIZE = 128
N_CORES = 8
TG = 4           # t-groups of 512
QG = 512
EXP_BIAS = -10.0
SCALE = 1.0 / float(np.sqrt(HSIZE))

_TRACE = os.environ.get("BASS_KERNEL_TRACE", "0") == "1"


def _patch_tile_drain():
    """walrus in this toolchain allows at most one sync-wait per instruction;
    TileContext's tail drain aggregates the whole global clock onto one Drain.
    Split it: one Drain per pending proc, each with a single wait."""
    if getattr(tile.TileContext, "_drain_patched", False):
        return

    def _drain_and_barrier(self, tick_clock, wait_clock):
        nc = self.nc
        gc = tick_clock.global_clock
        n = len(gc)
        for p in range(n):
            if gc[p] > 0:
                vc = VectorClock([gc[p] if i == p else 0 for i in range(n)])
                di = nc.sync.drain()
                wait_clock.add_sem_waits(di.ins, ScopedClock({None: vc}))
        nc.all_engine_barrier()
        popped = nc._tile_sem_poison_stack.pop()
        assert popped is self._sem_poison
        nc.clear_and_free_semaphores(list(self.sems.allocated().values()))
        nc.all_engine_barrier()

    tile.TileContext._drain_and_barrier = _drain_and_barrier
    tile.TileContext._drain_patched = True


def _install_ntff_hook():
    """Wire the axon NTFF profiling hook this image leaves unwired (the agent
    image's antenv lacks axon_hooks). Only needed when tracing."""
    import antenv

    if getattr(antenv, "axon_hooks", None) is not None:
        return
    mod = types.ModuleType("antenv.axon_hooks")
    mod._hook = None
    mod.set_axon_ntff_profile_hook = lambda h: setattr(mod, "_hook", h)
    mod.get_axon_ntff_profile_hook = lambda: mod._hook
    sys.modules["antenv.axon_hooks"] = mod
    antenv.axon_hooks = mod
    if "/root/.axon_site" not in sys.path:
        sys.path.insert(0, "/root/.axon_site")
    try:
        from trn_agent_boot.trn_boot import _ntff_profile_via_ctypes

        hook = _ntff_profile_via_ctypes("/opt/axon/libaxon_pjrt.so")
        if hook is not None:
            mod.set_axon_ntff_profile_hook(hook)
        import concourse.bass_utils as bu

        bu.upload_artifacts = lambda d: d
    except Exception:
        pass


def build_nc():
    _patch_tile_drain()
    nc = bacc.Bacc(None, target_bir_lowering=False)

    xT = nc.dram_tensor("xT", [C, T], F32R, kind="ExternalInput")
    w = nc.dram_tensor("w", [C, 6 * HSIZE * 2], F32R, kind="ExternalInput")  # [C,1536]
    wp = nc.dram_tensor("wp", [HPC * HSIZE, T], F32R, kind="ExternalInput")  # [512,T]
    c1d = nc.dram_tensor("c1", [P, T], F32R, kind="ExternalInput")
    c2d = nc.dram_tensor("c2", [P, T], F32R, kind="ExternalInput")
    mkd = nc.dram_tensor("mk", [4, P, QG], F32R, kind="ExternalInput")
    onesd = nc.dram_tensor("ones_col", [P, 1], F32R, kind="ExternalInput")
    onesrd = nc.dram_tensor("ones_row", [1, P], F32R, kind="ExternalInput")
    outT = nc.dram_tensor("outT", [T, T], F32, kind="ExternalOutput")  # [oc, t]

    xTr = xT.rearrange("(cc p) t -> p cc t", p=P)      # [128,16,2048]
    wr = w.rearrange("(cc p) j -> p cc j", p=P)        # [128,16,1536]
    wpr = wp.rearrange("(hc p) t -> p hc t", p=P)      # [128,4,2048]
    mkr = mkd.rearrange("s p q -> p s q")              # [128,4,512]

    def r(ap):
        return ap

    with tile.TileContext(nc) as tc, nc.allow_low_precision(
        reason="f32r storage is the intended reduced-precision matmul format"
    ):
        with (
            tc.tile_pool(name="const", bufs=1) as constp,
            tc.tile_pool(name="wpool", bufs=1) as wpool,
            tc.tile_pool(name="xtp", bufs=5) as xtp,
            tc.tile_pool(name="qk", bufs=1) as qkres,
            tc.tile_pool(name="vres", bufs=1) as vresp,
            tc.tile_pool(name="work", bufs=6) as work,
            tc.tile_pool(name="pwork", bufs=6) as pwork,
            tc.tile_pool(name="rp", bufs=2) as rpool,
            tc.tile_pool(name="mm", bufs=4, space="PSUM") as mmp,
            tc.tile_pool(name="yt", bufs=2, space="PSUM") as ytp,
            tc.tile_pool(name="lp", bufs=2, space="PSUM") as lpp,
            tc.tile_pool(name="dram", bufs=1, space="DRAM") as dramp,
        ):
            c1 = constp.tile([P, T], F32R, tag="c1")
            c2 = constp.tile([P, T], F32R, tag="c2")
            mk = constp.tile([P, 4, QG], F32R, tag="mk")
            ones_c = constp.tile([P, 1], F32R, tag="onc")
            ones_r = constp.tile([1, P], F32R, tag="onr")
            nc.sync.dma_start(c1[:], c1d[:])
            nc.sync.dma_start(c2[:], c2d[:])
            nc.sync.dma_start(mk[:], mkr)
            nc.sync.dma_start(ones_c[:], onesd[:])
            ebias = constp.tile([P, 1], F32, tag="ebias")
            nc.gpsimd.memset(ebias[:], EXP_BIAS)
            nc.sync.dma_start(ones_r[:], onesrd[:])

            yt_dram = dramp.tile([HPC * HSIZE, T], F32R)  # [512, 2048] spill

            for pair in range(2):
                w_sb = wpool.tile([P, 16, 768], F32R, tag="w")
                nc.sync.dma_start(w_sb[:], wr[:, :, pair * 768:(pair + 1) * 768])

                q_sb = [qkres.tile([P, T], F32R, tag=f"q{h}", name=f"q{h}") for h in range(2)]
                k_sb = [qkres.tile([P, T], F32R, tag=f"k{h}", name=f"k{h}") for h in range(2)]
                v_sb = vresp.tile([P, 16, 256], F32R, tag="v")

                # ---- QKV projection for this pair ----
                for tg in range(TG):
                    xts = []
                    for ch in range(4):
                        xt = xtp.tile([P, 4, QG], F32R, tag="xt")
                        nc.sync.dma_start(
                            xt[:],
                            xTr[:, ch * 4:(ch + 1) * 4, tg * QG:(tg + 1) * QG],
                        )
                        xts.append(xt)

                    # q/k: 4 j-tiles (q_h0, q_h1, k_h0, k_h1), N=512,
                    # in two passes of 2 concurrent psums to keep mm-pool slack
                    def rope(j, psum):
                        dst = (q_sb if j < 2 else k_sb)[j % 2]
                        dsl = dst[:, tg * QG:(tg + 1) * QG]
                        qraw = work.tile([P, QG], F32R, tag="tmp", name="qraw")
                        nc.scalar.activation(qraw[:], psum[:], AF.Copy)
                        qsw = work.tile([P, QG], F32R, tag="tmp", name="qsw")
                        nc.gpsimd.dma_start(qsw[0:64, :], qraw[64:128, :])
                        nc.gpsimd.dma_start(qsw[64:128, :], qraw[0:64, :])
                        t2 = work.tile([P, QG], F32R, tag="tmp", name="t2")
                        c1s = c1[:, tg * QG:(tg + 1) * QG]
                        c2s = c2[:, tg * QG:(tg + 1) * QG]
                        nc.vector.tensor_mul(dsl, qraw[:], c1s)
                        nc.vector.tensor_mul(t2[:], qsw[:], c2s)
                        nc.vector.tensor_add(dsl, dsl, t2[:])

                    for jp in range(2):
                        psq = [mmp.tile([P, QG], F32, tag="mm", name=f"psq{j}")
                               for j in range(2)]
                        for cc in range(16):
                            xt = xts[cc // 4][:, cc % 4, :]
                            for j in range(2):
                                nc.tensor.matmul(
                                    psq[j][:],
                                    r(w_sb[:, cc, (jp * 2 + j) * 128:(jp * 2 + j + 1) * 128]),
                                    r(xt),
                                    start=(cc == 0),
                                    stop=(cc == 15),
                                )
                        rope(jp * 2 + 0, psq[0])
                        rope(jp * 2 + 1, psq[1])
                    # v: 4 t-tiles in this tg, N=256 (both heads' v); yt pool is idle here
                    for tt in range(4):
                        psv = ytp.tile([P, 256], F32, tag="yt", name="psv")
                        for cc in range(16):
                            nc.tensor.matmul(
                                psv[:],
                                r(xts[cc // 4][:, cc % 4, tt * 128:(tt + 1) * 128]),
                                r(w_sb[:, cc, 512:768]),
                                start=(cc == 0),
                                stop=(cc == 15),
                            )
                        nc.vector.tensor_copy(v_sb[:, tg * 4 + tt, :], psv[:])

                # ---- attention for the pair's 2 heads ----
                for h in range(2):
                    hg = pair * 2 + h
                    for qg in range(TG):
                        n_kt = 4 * qg + 4
                        LA = 2
                        ps_y = ytp.tile([P, QG], F32, tag="yt")
                        ps_l = lpp.tile([1, QG], F32, tag="l")
                        p_tiles = {}

                        def emit_s(kt):
                            ps_s = mmp.tile([P, QG], F32, tag="mm", name="ps_s")
                            nc.tensor.matmul(
                                ps_s[:],
                                r(k_sb[h][:, kt * 128:(kt + 1) * 128]),
                                r(q_sb[h][:, qg * QG:(qg + 1) * QG]),
                                start=True,
                                stop=True,
                            )
                            p_sb = pwork.tile([P, QG], F32R, tag="p", name="p_sb")
                            nc.scalar.activation(
                                p_sb[:], ps_s[:], AF.Exp, bias=ebias[:], scale=SCALE
                            )
                            if kt >= 4 * qg:
                                s = kt - 4 * qg
                                nc.vector.tensor_mul(p_sb[:], p_sb[:], mk[:, s, :])
                            p_tiles[kt] = p_sb

                        def emit_av(kt):
                            p_sb = p_tiles.pop(kt)
                            nc.tensor.matmul(
                                ps_y[:],
                                r(v_sb[:, kt, h * 128:(h + 1) * 128]),
                                r(p_sb[:]),
                                start=(kt == 0),
                                stop=(kt == n_kt - 1),
                            )
                            nc.tensor.matmul(
                                ps_l[:],
                                r(ones_c[:]),
                                r(p_sb[:]),
                                start=(kt == 0),
                                stop=(kt == n_kt - 1),
                            )

                        for kt in range(n_kt + LA):
                            if kt < n_kt:
                                emit_s(kt)
                            if kt >= LA:
                                emit_av(kt - LA)
                        r_sb = rpool.tile([1, QG], F32R, tag="r")
                        nc.vector.reciprocal(r_sb[:], ps_l[:])
                        ps_r = mmp.tile([P, QG], F32, tag="mm")
                        nc.tensor.matmul(
                            ps_r[:], r(ones_r[:]), r(r_sb[:]), start=True, stop=True
                        )
                        r128 = pwork.tile([P, QG], F32, tag="p")
                        nc.scalar.activation(r128[:], ps_r[:], AF.Copy)
                        stage = work.tile([P, QG], F32R, tag="tmp")
                        nc.vector.tensor_mul(stage[:], ps_y[:], r128[:])
                        nc.sync.dma_start(
                            yt_dram[hg * 128:(hg + 1) * 128, qg * QG:(qg + 1) * QG],
                            stage[:],
                        )

            # ---- output projection: outT[oc, t] = wp^T-slice @ yT ----
            wp_sb = wpool.tile([P, 4, T], F32R, tag="w")
            nc.sync.dma_start(wp_sb[:], wpr)
            ytr = yt_dram.rearrange("(hc p) t -> p hc t", p=P)
            yt_sb = [xtp.tile([P, T], F32R, tag="xt", name=f"yt{i}") for i in range(4)]
            for hc in range(4):
                nc.sync.dma_start(yt_sb[hc][:], ytr[:, hc, :])
            for oc in range(16):
                for tg in range(TG):
                    ps_o = mmp.tile([P, QG], F32, tag="mm")
                    for hc in range(4):
                        nc.tensor.matmul(
                            ps_o[:],
                            r(wp_sb[:, hc, oc * 128:(oc + 1) * 128]),
                            r(yt_sb[hc][:, tg * QG:(tg + 1) * QG]),
                            start=(hc == 0),
                            stop=(hc == 3),
                        )
                    stage = work.tile([P, QG], F32, tag="tmp")
                    nc.vector.tensor_copy(stage[:], ps_o[:])
                    nc.sync.dma_start(
                        outT[oc * 128:(oc + 1) * 128, tg * QG:(tg + 1) * QG],
                        stage[:],
                    )

    nc.finalize()
    return nc


def _host_inputs(x, freqs_cis, w_attn, w_proj):
    """Build the 8 per-core input maps."""
    x = np.asarray(x, dtype=np.float32)
    freqs_cis = np.asarray(freqs_cis, dtype=np.float32)
    w_attn = np.asarray(w_attn, dtype=np.float32)
    w_proj = np.asarray(w_proj, dtype=np.float32)

    B = x.shape[0]
    perm = np.concatenate([np.arange(0, HSIZE, 2), np.arange(1, HSIZE, 2)])

    cos = np.ascontiguousarray(freqs_cis[:, :, 0].T)  # [64, T]
    sin = np.ascontiguousarray(freqs_cis[:, :, 1].T)
    c1 = np.concatenate([cos, cos], axis=0)           # [128, T]
    c2 = np.concatenate([-sin, sin], axis=0)

    kk = np.arange(P)[:, None]
    ccol = np.arange(QG)[None, :]
    mk = np.stack(
        [(ccol >= s * 128 + kk).astype(np.float32) for s in range(4)], axis=0
    )  # [4,128,512]

    ones_col = np.ones((P, 1), np.float32)
    ones_row = np.ones((1, P), np.float32)

    xT = [np.ascontiguousarray(x[b].T) for b in range(B)]

    in_maps = []
    for core in range(N_CORES):
        b, g = core // 4, core % 4
        blocks = []
        for pairp in range(2):
            for off in (0, C, 2 * C):  # q, k, v origins in w_attn
                for hh in range(2):
                    hglob = 4 * g + 2 * pairp + hh
                    cols = w_attn[:, off + hglob * HSIZE: off + (hglob + 1) * HSIZE]
                    if off != 2 * C:  # permute q and k, not v
                        cols = cols[:, perm]
                    blocks.append(cols)
        wcore = np.ascontiguousarray(np.concatenate(blocks, axis=1))  # [C, 1536]
        wpcore = np.ascontiguousarray(w_proj[g * 512:(g + 1) * 512, :])
        in_maps.append(
            {
                "xT": xT[b],
                "w": wcore,
                "wp": wpcore,
                "c1": c1,
                "c2": c2,
                "mk": mk,
                "ones_col": ones_col,
                "ones_row": ones_row,
            }
        )
    return in_maps


_LAST_RESULT = {}


def kernel(x, freqs_cis, w_attn, w_proj):
    if _TRACE:
        _install_ntff_hook()
    in_maps = _host_inputs(x, freqs_cis, w_attn, w_proj)
    nc = build_nc()
    res = run_bass_kernel_spmd(
        nc, in_maps, core_ids=list(range(N_CORES)), trace=_TRACE
    )
    _LAST_RESULT["res"] = res

    B = x.shape[0]
    out = np.zeros((B, T, C), dtype=np.float32)
    for core in range(N_CORES):
        b = core // 4
        out[b] += res.results[core]["outT"].T
    return out


# revision 8
# speedup vs baseline: 1.0867x; 1.0867x over previous
"""Causal self-attention (dense transformer block) for 8 Trainium2 NeuronCores.

Sharding: DP over batch (2) x TP over heads (4 groups of 4 heads) = 8 cores.
Per core: column-parallel QKV projection (4 heads), RoPE, causal
flash-attention (no-max-subtraction softmax with constant bias), row-parallel
output projection producing a partial [oc, t] result; host sums the 4 TP
partials per batch and transposes back.

Device matmuls run as float32r (full-rate fp32 storage, ~tf32 accuracy).

Self-contained: hardcodes shapes, builds/compiles/runs the Bass kernel via
run_bass_kernel_spmd on cores 0-7.
"""

import os
import sys
import types

sys.path.insert(0, "/opt/trn_rl_repo")

import numpy as np

import concourse.bass as bass
import concourse.mybir as mybir
import concourse.tile as tile
from concourse import bacc
from concourse.bass_utils import run_bass_kernel_spmd
from concourse.vector_clock import ScopedClock, VectorClock

F32 = mybir.dt.float32
F32R = mybir.dt.float32r
AF = mybir.ActivationFunctionType
ALU = mybir.AluOpType

P = 128
T = 2048
C = 2048
NH = 16          # total heads
HPC = 4          # heads per core
HSIZE = 128
N_CORES = 8
TG = 4           # t-groups of 512
QG = 512
EXP_BIAS = -10.0
SCALE = 1.0 / float(np.sqrt(HSIZE))

_TRACE = os.environ.get("BASS_KERNEL_TRACE", "0") == "1"


def _patch_tile_drain():
    """walrus in this toolchain allows at most one sync-wait per instruction;
    TileContext's tail drain aggregates the whole global clock onto one Drain.
    Split it: one Drain per pending proc, each with a single wait."""
    if getattr(tile.TileContext, "_drain_patched", False):
        return

    def _drain_and_barrier(self, tick_clock, wait_clock):
        nc = self.nc
        gc = tick_clock.global_clock
        n = len(gc)
        for p in range(n):
            if gc[p] > 0:
                vc = VectorClock([gc[p] if i == p else 0 for i in range(n)])
                di = nc.sync.drain()
                wait_clock.add_sem_waits(di.ins, ScopedClock({None: vc}))
        nc.all_engine_barrier()
        popped = nc._tile_sem_poison_stack.pop()
        assert popped is self._sem_poison
        nc.clear_and_free_semaphores(list(self.sems.allocated().values()))
        nc.all_engine_barrier()

    tile.TileContext._drain_and_barrier = _drain_and_barrier
    tile.TileContext._drain_patched = True


def _install_ntff_hook():
    """Wire the axon NTFF profiling hook this image leaves unwired (the agent
    image's antenv lacks axon_hooks). Only needed when tracing."""
    import antenv

    if getattr(antenv, "axon_hooks", None) is not None:
        return
    mod = types.ModuleType("antenv.axon_hooks")
    mod._hook = None
    mod.set_axon_ntff_profile_hook = lambda h: setattr(mod, "_hook", h)
    mod.get_axon_ntff_profile_hook = lambda: mod._hook
    sys.modules["antenv.axon_hooks"] = mod
    antenv.axon_hooks = mod
    if "/root/.axon_site" not in sys.path:
        sys.path.insert(0, "/root/.axon_site")
    try:
        from trn_agent_boot.trn_boot import _ntff_profile_via_ctypes

        hook = _ntff_profile_via_ctypes("/opt/axon/libaxon_pjrt.so")
        if hook is not None:
            mod.set_axon_ntff_profile_hook(hook)
        import concourse.bass_utils as bu

        bu.upload_artifacts = lambda d: d
    except Exception:
        pass


def build_nc():
    _patch_tile_drain()
    nc = bacc.Bacc(None, target_bir_lowering=False)

    xT = nc.dram_tensor("xT", [C, T], F32R, kind="ExternalInput")
    w = nc.dram_tensor("w", [C, 6 * HSIZE * 2], F32R, kind="ExternalInput")  # [C,1536]
    wp = nc.dram_tensor("wp", [HPC * HSIZE, T], F32R, kind="ExternalInput")  # [512,T]
    c1d = nc.dram_tensor("c1", [P, T], F32R, kind="ExternalInput")
    c2d = nc.dram_tensor("c2", [P, T], F32R, kind="ExternalInput")
    mkd = nc.dram_tensor("mk", [4, P, QG], F32R, kind="ExternalInput")
    onesd = nc.dram_tensor("ones_col", [P, 1], F32R, kind="ExternalInput")
    onesrd = nc.dram_tensor("ones_row", [1, P], F32R, kind="ExternalInput")
    outT = nc.dram_tensor("outT", [T, T], F32, kind="ExternalOutput")  # [oc, t]

    xTr = xT.rearrange("(cc p) t -> p cc t", p=P)      # [128,16,2048]
    wr = w.rearrange("(cc p) j -> p cc j", p=P)        # [128,16,1536]
    wpr = wp.rearrange("(hc p) t -> p hc t", p=P)      # [128,4,2048]
    mkr = mkd.rearrange("s p q -> p s q")              # [128,4,512]

    def r(ap):
        return ap

    with tile.TileContext(nc) as tc, nc.allow_low_precision(
        reason="f32r storage is the intended reduced-precision matmul format"
    ):
        with (
            tc.tile_pool(name="const", bufs=1) as constp,
            tc.tile_pool(name="wpool", bufs=1) as wpool,
            tc.tile_pool(name="xtp", bufs=5) as xtp,
            tc.tile_pool(name="qk", bufs=1) as qkres,
            tc.tile_pool(name="vres", bufs=1) as vresp,
            tc.tile_pool(name="work", bufs=6) as work,
            tc.tile_pool(name="pwork", bufs=6) as pwork,
            tc.tile_pool(name="rp", bufs=2) as rpool,
            tc.tile_pool(name="mm", bufs=4, space="PSUM") as mmp,
            tc.tile_pool(name="yt", bufs=2, space="PSUM") as ytp,
            tc.tile_pool(name="lp", bufs=2, space="PSUM") as lpp,
            tc.tile_pool(name="dram", bufs=1, space="DRAM") as dramp,
        ):
            c1 = constp.tile([P, T], F32R, tag="c1")
            c2 = constp.tile([P, T], F32R, tag="c2")
            mk = constp.tile([P, 4, QG], F32R, tag="mk")
            ones_c = constp.tile([P, 1], F32R, tag="onc")
            ones_r = constp.tile([1, P], F32R, tag="onr")
            nc.sync.dma_start(c1[:], c1d[:])
            nc.sync.dma_start(c2[:], c2d[:])
            nc.sync.dma_start(mk[:], mkr)
            nc.sync.dma_start(ones_c[:], onesd[:])
            ebias = constp.tile([P, 1], F32, tag="ebias")
            nc.gpsimd.memset(ebias[:], EXP_BIAS)
            nc.sync.dma_start(ones_r[:], onesrd[:])

            yt_dram = dramp.tile([HPC * HSIZE, T], F32R)  # [512, 2048] spill

            for pair in range(2):
                w_sb = wpool.tile([P, 16, 768], F32R, tag="w")
                nc.sync.dma_start(w_sb[:], wr[:, :, pair * 768:(pair + 1) * 768])

                q_sb = [qkres.tile([P, T], F32R, tag=f"q{h}", name=f"q{h}") for h in range(2)]
                k_sb = [qkres.tile([P, T], F32R, tag=f"k{h}", name=f"k{h}") for h in range(2)]
                v_sb = vresp.tile([P, 16, 256], F32R, tag="v")

                # ---- QKV projection for this pair ----
                for tg in range(TG):
                    xts = []
                    for ch in range(4):
                        xt = xtp.tile([P, 4, QG], F32R, tag="xt")
                        nc.sync.dma_start(
                            xt[:],
                            xTr[:, ch * 4:(ch + 1) * 4, tg * QG:(tg + 1) * QG],
                        )
                        xts.append(xt)

                    # q/k: 4 j-tiles (q_h0, q_h1, k_h0, k_h1), N=512,
                    # in two passes of 2 concurrent psums to keep mm-pool slack
                    def rope(j, psum):
                        dst = (q_sb if j < 2 else k_sb)[j % 2]
                        dsl = dst[:, tg * QG:(tg + 1) * QG]
                        qraw = work.tile([P, QG], F32R, tag="tmp", name="qraw")
                        nc.scalar.activation(qraw[:], psum[:], AF.Copy)
                        qsw = work.tile([P, QG], F32R, tag="tmp", name="qsw")
                        nc.gpsimd.dma_start(qsw[0:64, :], qraw[64:128, :])
                        nc.gpsimd.dma_start(qsw[64:128, :], qraw[0:64, :])
                        t2 = work.tile([P, QG], F32R, tag="tmp", name="t2")
                        c1s = c1[:, tg * QG:(tg + 1) * QG]
                        c2s = c2[:, tg * QG:(tg + 1) * QG]
                        nc.vector.tensor_mul(dsl, qraw[:], c1s)
                        nc.vector.tensor_mul(t2[:], qsw[:], c2s)
                        nc.vector.tensor_add(dsl, dsl, t2[:])

                    for jp in range(2):
                        psq = [mmp.tile([P, QG], F32, tag="mm", name=f"psq{j}")
                               for j in range(2)]
                        for cc in range(16):
                            xt = xts[cc // 4][:, cc % 4, :]
                            for j in range(2):
                                nc.tensor.matmul(
                                    psq[j][:],
                                    r(w_sb[:, cc, (jp * 2 + j) * 128:(jp * 2 + j + 1) * 128]),
                                    r(xt),
                                    start=(cc == 0),
                                    stop=(cc == 15),
                                )
                        rope(jp * 2 + 0, psq[0])
                        rope(jp * 2 + 1, psq[1])
                    # v: 4 t-tiles in this tg, N=256 (both heads' v); yt pool is idle here
                    for tt in range(4):
                        psv = ytp.tile([P, 256], F32, tag="yt", name="psv")
                        for cc in range(16):
                            nc.tensor.matmul(
                                psv[:],
                                r(xts[cc // 4][:, cc % 4, tt * 128:(tt + 1) * 128]),
                                r(w_sb[:, cc, 512:768]),
                                start=(cc == 0),
                                stop=(cc == 15),
                            )
                        nc.vector.tensor_copy(v_sb[:, tg * 4 + tt, :], psv[:])

                # ---- attention for the pair's 2 heads ----
                pending_norm = []

                def emit_norm():
                    hg_, qg_, ps_y_, ps_l_ = pending_norm.pop(0)
                    r_sb = rpool.tile([1, QG], F32R, tag="r", name="r_sb")
                    nc.vector.reciprocal(r_sb[:], ps_l_[:])
                    ps_r = mmp.tile([P, QG], F32, tag="mm", name="ps_r")
                    nc.tensor.matmul(
                        ps_r[:], r(ones_r[:]), r(r_sb[:]), start=True, stop=True
                    )
                    r128 = pwork.tile([P, QG], F32, tag="p", name="r128")
                    nc.scalar.activation(r128[:], ps_r[:], AF.Copy)
                    stage = work.tile([P, QG], F32R, tag="tmp", name="stage")
                    nc.vector.tensor_mul(stage[:], ps_y_[:], r128[:])
                    nc.sync.dma_start(
                        yt_dram[hg_ * 128:(hg_ + 1) * 128, qg_ * QG:(qg_ + 1) * QG],
                        stage[:],
                    )

                for h in range(2):
                    hg = pair * 2 + h
                    for qg in range(TG):
                        n_kt = 4 * qg + 4
                        LA = 2
                        ps_y = ytp.tile([P, QG], F32, tag="yt")
                        ps_l = lpp.tile([1, QG], F32, tag="l")
                        p_tiles = {}

                        def emit_s(kt):
                            ps_s = mmp.tile([P, QG], F32, tag="mm", name="ps_s")
                            nc.tensor.matmul(
                                ps_s[:],
                                r(k_sb[h][:, kt * 128:(kt + 1) * 128]),
                                r(q_sb[h][:, qg * QG:(qg + 1) * QG]),
                                start=True,
                                stop=True,
                            )
                            p_sb = pwork.tile([P, QG], F32R, tag="p", name="p_sb")
                            nc.scalar.activation(
                                p_sb[:], ps_s[:], AF.Exp, bias=ebias[:], scale=SCALE
                            )
                            if kt >= 4 * qg:
                                s = kt - 4 * qg
                                nc.vector.tensor_mul(p_sb[:], p_sb[:], mk[:, s, :])
                            p_tiles[kt] = p_sb

                        def emit_av(kt):
                            p_sb = p_tiles.pop(kt)
                            nc.tensor.matmul(
                                ps_y[:],
                                r(v_sb[:, kt, h * 128:(h + 1) * 128]),
                                r(p_sb[:]),
                                start=(kt == 0),
                                stop=(kt == n_kt - 1),
                            )
                            nc.tensor.matmul(
                                ps_l[:],
                                r(ones_c[:]),
                                r(p_sb[:]),
                                start=(kt == 0),
                                stop=(kt == n_kt - 1),
                            )

                        for kt in range(n_kt + LA):
                            if kt < n_kt:
                                emit_s(kt)
                            if kt == 3 and pending_norm:
                                emit_norm()
                            if kt >= LA:
                                emit_av(kt - LA)
                        pending_norm.append((hg, qg, ps_y, ps_l))

                while pending_norm:
                    emit_norm()

            # ---- output projection: outT[oc, t] = wp^T-slice @ yT ----
            wp_sb = wpool.tile([P, 4, T], F32R, tag="w")
            nc.sync.dma_start(wp_sb[:], wpr)
            ytr = yt_dram.rearrange("(hc p) t -> p hc t", p=P)
            yt_sb = [xtp.tile([P, T], F32R, tag="xt", name=f"yt{i}") for i in range(4)]
            for hc in range(4):
                nc.sync.dma_start(yt_sb[hc][:], ytr[:, hc, :])
            for oc in range(16):
                for tg in range(TG):
                    ps_o = mmp.tile([P, QG], F32, tag="mm")
                    for hc in range(4):
                        nc.tensor.matmul(
                            ps_o[:],
                            r(wp_sb[:, hc, oc * 128:(oc + 1) * 128]),
                            r(yt_sb[hc][:, tg * QG:(tg + 1) * QG]),
                            start=(hc == 0),
                            stop=(hc == 3),
                        )
                    stage = work.tile([P, QG], F32, tag="tmp")
                    nc.vector.tensor_copy(stage[:], ps_o[:])
                    nc.sync.dma_start(
                        outT[oc * 128:(oc + 1) * 128, tg * QG:(tg + 1) * QG],
                        stage[:],
                    )

    nc.finalize()
    return nc


def _host_inputs(x, freqs_cis, w_attn, w_proj):
    """Build the 8 per-core input maps."""
    x = np.asarray(x, dtype=np.float32)
    freqs_cis = np.asarray(freqs_cis, dtype=np.float32)
    w_attn = np.asarray(w_attn, dtype=np.float32)
    w_proj = np.asarray(w_proj, dtype=np.float32)

    B = x.shape[0]
    perm = np.concatenate([np.arange(0, HSIZE, 2), np.arange(1, HSIZE, 2)])

    cos = np.ascontiguousarray(freqs_cis[:, :, 0].T)  # [64, T]
    sin = np.ascontiguousarray(freqs_cis[:, :, 1].T)
    c1 = np.concatenate([cos, cos], axis=0)           # [128, T]
    c2 = np.concatenate([-sin, sin], axis=0)

    kk = np.arange(P)[:, None]
    ccol = np.arange(QG)[None, :]
    mk = np.stack(
        [(ccol >= s * 128 + kk).astype(np.float32) for s in range(4)], axis=0
    )  # [4,128,512]

    ones_col = np.ones((P, 1), np.float32)
    ones_row = np.ones((1, P), np.float32)

    xT = [np.ascontiguousarray(x[b].T) for b in range(B)]

    in_maps = []
    for core in range(N_CORES):
        b, g = core // 4, core % 4
        blocks = []
        for pairp in range(2):
            for off in (0, C, 2 * C):  # q, k, v origins in w_attn
                for hh in range(2):
                    hglob = 4 * g + 2 * pairp + hh
                    cols = w_attn[:, off + hglob * HSIZE: off + (hglob + 1) * HSIZE]
                    if off != 2 * C:  # permute q and k, not v
                        cols = cols[:, perm]
                    blocks.append(cols)
        wcore = np.ascontiguousarray(np.concatenate(blocks, axis=1))  # [C, 1536]
        wpcore = np.ascontiguousarray(w_proj[g * 512:(g + 1) * 512, :])
        in_maps.append(
            {
                "xT": xT[b],
                "w": wcore,
                "wp": wpcore,
                "c1": c1,
                "c2": c2,
                "mk": mk,
                "ones_col": ones_col,
                "ones_row": ones_row,
            }
        )
    return in_maps


_LAST_RESULT = {}


def kernel(x, freqs_cis, w_attn, w_proj):
    if _TRACE:
        _install_ntff_hook()
    in_maps = _host_inputs(x, freqs_cis, w_attn, w_proj)
    nc = build_nc()
    res = run_bass_kernel_spmd(
        nc, in_maps, core_ids=list(range(N_CORES)), trace=_TRACE
    )
    _LAST_RESULT["res"] = res

    B = x.shape[0]
    out = np.zeros((B, T, C), dtype=np.float32)
    for core in range(N_CORES):
        b = core // 4
        out[b] += res.results[core]["outT"].T
    return out


# revision 9
# speedup vs baseline: 1.1254x; 1.0357x over previous
"""Causal self-attention (dense transformer block) for 8 Trainium2 NeuronCores.

Sharding: DP over batch (2) x TP over heads (4 groups of 4 heads) = 8 cores.
Per core: column-parallel QKV projection (4 heads), RoPE, causal
flash-attention (no-max-subtraction softmax with constant bias), row-parallel
output projection producing a partial [oc, t] result; host sums the 4 TP
partials per batch and transposes back.

Device matmuls run as float32r (full-rate fp32 storage, ~tf32 accuracy).

Self-contained: hardcodes shapes, builds/compiles/runs the Bass kernel via
run_bass_kernel_spmd on cores 0-7.
"""

import os
import sys
import types

sys.path.insert(0, "/opt/trn_rl_repo")

import numpy as np

import concourse.bass as bass
import concourse.mybir as mybir
import concourse.tile as tile
from concourse import bacc
from concourse.bass_utils import run_bass_kernel_spmd
from concourse.vector_clock import ScopedClock, VectorClock

F32 = mybir.dt.float32
F32R = mybir.dt.float32r
AF = mybir.ActivationFunctionType
ALU = mybir.AluOpType

P = 128
T = 2048
C = 2048
NH = 16          # total heads
HPC = 4          # heads per core
HSIZE = 128
N_CORES = 8
TG = 4           # t-groups of 512
QG = 512
EXP_BIAS = -10.0
SCALE = 1.0 / float(np.sqrt(HSIZE))

_TRACE = os.environ.get("BASS_KERNEL_TRACE", "0") == "1"


def _patch_tile_drain():
    """walrus in this toolchain allows at most one sync-wait per instruction;
    TileContext's tail drain aggregates the whole global clock onto one Drain.
    Split it: one Drain per pending proc, each with a single wait."""
    if getattr(tile.TileContext, "_drain_patched", False):
        return

    def _drain_and_barrier(self, tick_clock, wait_clock):
        nc = self.nc
        gc = tick_clock.global_clock
        n = len(gc)
        for p in range(n):
            if gc[p] > 0:
                vc = VectorClock([gc[p] if i == p else 0 for i in range(n)])
                di = nc.sync.drain()
                wait_clock.add_sem_waits(di.ins, ScopedClock({None: vc}))
        nc.all_engine_barrier()
        popped = nc._tile_sem_poison_stack.pop()
        assert popped is self._sem_poison
        nc.clear_and_free_semaphores(list(self.sems.allocated().values()))
        nc.all_engine_barrier()

    tile.TileContext._drain_and_barrier = _drain_and_barrier
    tile.TileContext._drain_patched = True


def _install_ntff_hook():
    """Wire the axon NTFF profiling hook this image leaves unwired (the agent
    image's antenv lacks axon_hooks). Only needed when tracing."""
    import antenv

    if getattr(antenv, "axon_hooks", None) is not None:
        return
    mod = types.ModuleType("antenv.axon_hooks")
    mod._hook = None
    mod.set_axon_ntff_profile_hook = lambda h: setattr(mod, "_hook", h)
    mod.get_axon_ntff_profile_hook = lambda: mod._hook
    sys.modules["antenv.axon_hooks"] = mod
    antenv.axon_hooks = mod
    if "/root/.axon_site" not in sys.path:
        sys.path.insert(0, "/root/.axon_site")
    try:
        from trn_agent_boot.trn_boot import _ntff_profile_via_ctypes

        hook = _ntff_profile_via_ctypes("/opt/axon/libaxon_pjrt.so")
        if hook is not None:
            mod.set_axon_ntff_profile_hook(hook)
        import concourse.bass_utils as bu

        bu.upload_artifacts = lambda d: d
    except Exception:
        pass


def build_nc():
    _patch_tile_drain()
    nc = bacc.Bacc(None, target_bir_lowering=False)

    xT = nc.dram_tensor("xT", [C, T], F32R, kind="ExternalInput")
    w = nc.dram_tensor("w", [C, 6 * HSIZE * 2], F32R, kind="ExternalInput")  # [C,1536]
    wp = nc.dram_tensor("wp", [HPC * HSIZE, T], F32R, kind="ExternalInput")  # [512,T]
    c1d = nc.dram_tensor("c1", [P, T], F32R, kind="ExternalInput")
    c2d = nc.dram_tensor("c2", [P, T], F32R, kind="ExternalInput")
    mkd = nc.dram_tensor("mk", [4, P, QG], F32R, kind="ExternalInput")
    onesd = nc.dram_tensor("ones_col", [P, 1], F32R, kind="ExternalInput")
    onesrd = nc.dram_tensor("ones_row", [1, P], F32R, kind="ExternalInput")
    outT = nc.dram_tensor("outT", [T, T], F32, kind="ExternalOutput")  # [oc, t]

    xTr = xT.rearrange("(cc p) t -> p cc t", p=P)      # [128,16,2048]
    wr = w.rearrange("(cc p) j -> p cc j", p=P)        # [128,16,1536]
    wpr = wp.rearrange("(hc p) t -> p hc t", p=P)      # [128,4,2048]
    mkr = mkd.rearrange("s p q -> p s q")              # [128,4,512]

    def r(ap):
        return ap

    with tile.TileContext(nc) as tc, nc.allow_low_precision(
        reason="f32r storage is the intended reduced-precision matmul format"
    ):
        with (
            tc.tile_pool(name="const", bufs=1) as constp,
            tc.tile_pool(name="wpool", bufs=1) as wpool,
            tc.tile_pool(name="xtp", bufs=5) as xtp,
            tc.tile_pool(name="qk", bufs=1) as qkres,
            tc.tile_pool(name="vres", bufs=1) as vresp,
            tc.tile_pool(name="work", bufs=6) as work,
            tc.tile_pool(name="pwork", bufs=6) as pwork,
            tc.tile_pool(name="rp", bufs=2) as rpool,
            tc.tile_pool(name="mm", bufs=4, space="PSUM") as mmp,
            tc.tile_pool(name="yt", bufs=2, space="PSUM") as ytp,
            tc.tile_pool(name="lp", bufs=2, space="PSUM") as lpp,
            tc.tile_pool(name="dram", bufs=1, space="DRAM") as dramp,
        ):
            c1 = constp.tile([P, T], F32R, tag="c1")
            c2 = constp.tile([P, T], F32R, tag="c2")
            mk = constp.tile([P, 4, QG], F32R, tag="mk")
            ones_c = constp.tile([P, 1], F32R, tag="onc")
            ones_r = constp.tile([1, P], F32R, tag="onr")
            nc.sync.dma_start(c1[:], c1d[:])
            nc.sync.dma_start(c2[:], c2d[:])
            nc.sync.dma_start(mk[:], mkr)
            nc.sync.dma_start(ones_c[:], onesd[:])
            ebias = constp.tile([P, 1], F32, tag="ebias")
            nc.gpsimd.memset(ebias[:], EXP_BIAS)
            nc.sync.dma_start(ones_r[:], onesrd[:])

            yt_dram = dramp.tile([HPC * HSIZE, T], F32R)  # [512, 2048] spill

            for pair in range(2):
                w_sb = wpool.tile([P, 16, 768], F32R, tag="w")
                nc.sync.dma_start(w_sb[:], wr[:, :, pair * 768:(pair + 1) * 768])

                q_sb = [qkres.tile([P, T], F32R, tag=f"q{h}", name=f"q{h}") for h in range(2)]
                k_sb = [qkres.tile([P, T], F32R, tag=f"k{h}", name=f"k{h}") for h in range(2)]
                v_sb = vresp.tile([P, 16, 256], F32R, tag="v")

                # ---- QKV projection for this pair ----
                for tg in range(TG):
                    xts = []
                    for ch in range(4):
                        xt = xtp.tile([P, 4, QG], F32R, tag="xt")
                        nc.sync.dma_start(
                            xt[:],
                            xTr[:, ch * 4:(ch + 1) * 4, tg * QG:(tg + 1) * QG],
                        )
                        xts.append(xt)

                    # q/k: 4 j-tiles (q_h0, q_h1, k_h0, k_h1), N=512,
                    # in two passes of 2 concurrent psums to keep mm-pool slack
                    def rope(j, psum):
                        dst = (q_sb if j < 2 else k_sb)[j % 2]
                        dsl = dst[:, tg * QG:(tg + 1) * QG]
                        qraw = work.tile([P, QG], F32R, tag="tmp", name="qraw")
                        nc.scalar.activation(qraw[:], psum[:], AF.Copy)
                        qsw = work.tile([P, QG], F32R, tag="tmp", name="qsw")
                        nc.gpsimd.dma_start(qsw[0:64, :], qraw[64:128, :])
                        nc.gpsimd.dma_start(qsw[64:128, :], qraw[0:64, :])
                        t2 = work.tile([P, QG], F32R, tag="tmp", name="t2")
                        c1s = c1[:, tg * QG:(tg + 1) * QG]
                        c2s = c2[:, tg * QG:(tg + 1) * QG]
                        nc.vector.tensor_mul(dsl, qraw[:], c1s)
                        nc.vector.tensor_mul(t2[:], qsw[:], c2s)
                        nc.vector.tensor_add(dsl, dsl, t2[:])

                    for jp in range(2):
                        psq = [mmp.tile([P, QG], F32, tag="mm", name=f"psq{j}")
                               for j in range(2)]
                        for cc in range(16):
                            xt = xts[cc // 4][:, cc % 4, :]
                            for j in range(2):
                                nc.tensor.matmul(
                                    psq[j][:],
                                    r(w_sb[:, cc, (jp * 2 + j) * 128:(jp * 2 + j + 1) * 128]),
                                    r(xt),
                                    start=(cc == 0),
                                    stop=(cc == 15),
                                )
                        rope(jp * 2 + 0, psq[0])
                        rope(jp * 2 + 1, psq[1])
                    # v: 4 t-tiles in this tg, N=256 (both heads' v); yt pool is idle here
                    for tt in range(4):
                        psv = ytp.tile([P, 256], F32, tag="yt", name="psv")
                        for cc in range(16):
                            nc.tensor.matmul(
                                psv[:],
                                r(xts[cc // 4][:, cc % 4, tt * 128:(tt + 1) * 128]),
                                r(w_sb[:, cc, 512:768]),
                                start=(cc == 0),
                                stop=(cc == 15),
                            )
                        nc.vector.tensor_copy(v_sb[:, tg * 4 + tt, :], psv[:])

                # ---- attention for the pair's 2 heads ----
                pending_norm = []

                def emit_norm():
                    hg_, qg_, ps_y_, ps_l_ = pending_norm.pop(0)
                    r_f32 = rpool.tile([1, QG], F32, tag="rf", name="r_f32")
                    nc.vector.reciprocal_approx_fast(r_f32[:], ps_l_[:])
                    r_sb = rpool.tile([1, QG], F32R, tag="r", name="r_sb")
                    nc.scalar.activation(r_sb[:], r_f32[:], AF.Copy)
                    ps_r = mmp.tile([P, QG], F32, tag="mm", name="ps_r")
                    nc.tensor.matmul(
                        ps_r[:], r(ones_r[:]), r(r_sb[:]), start=True, stop=True
                    )
                    r128 = pwork.tile([P, QG], F32, tag="p", name="r128")
                    nc.scalar.activation(r128[:], ps_r[:], AF.Copy)
                    stage = work.tile([P, QG], F32R, tag="tmp", name="stage")
                    nc.vector.tensor_mul(stage[:], ps_y_[:], r128[:])
                    nc.sync.dma_start(
                        yt_dram[hg_ * 128:(hg_ + 1) * 128, qg_ * QG:(qg_ + 1) * QG],
                        stage[:],
                    )

                for h in range(2):
                    hg = pair * 2 + h
                    for qg in range(TG):
                        n_kt = 4 * qg + 4
                        LA = 2
                        ps_y = ytp.tile([P, QG], F32, tag="yt")
                        ps_l = lpp.tile([1, QG], F32, tag="l")
                        p_tiles = {}

                        def emit_s(kt):
                            ps_s = mmp.tile([P, QG], F32, tag="mm", name="ps_s")
                            nc.tensor.matmul(
                                ps_s[:],
                                r(k_sb[h][:, kt * 128:(kt + 1) * 128]),
                                r(q_sb[h][:, qg * QG:(qg + 1) * QG]),
                                start=True,
                                stop=True,
                            )
                            p_sb = pwork.tile([P, QG], F32R, tag="p", name="p_sb")
                            nc.scalar.activation(
                                p_sb[:], ps_s[:], AF.Exp, bias=ebias[:], scale=SCALE
                            )
                            if kt >= 4 * qg:
                                s = kt - 4 * qg
                                nc.gpsimd.tensor_mul(p_sb[:], p_sb[:], mk[:, s, :])
                            p_tiles[kt] = p_sb

                        def emit_av(kt):
                            p_sb = p_tiles.pop(kt)
                            nc.tensor.matmul(
                                ps_y[:],
                                r(v_sb[:, kt, h * 128:(h + 1) * 128]),
                                r(p_sb[:]),
                                start=(kt == 0),
                                stop=(kt == n_kt - 1),
                            )
                            nc.tensor.matmul(
                                ps_l[:],
                                r(ones_c[:]),
                                r(p_sb[:]),
                                start=(kt == 0),
                                stop=(kt == n_kt - 1),
                            )

                        for kt in range(n_kt + LA):
                            if kt < n_kt:
                                emit_s(kt)
                            if kt == 3 and pending_norm:
                                emit_norm()
                            if kt >= LA:
                                emit_av(kt - LA)
                        pending_norm.append((hg, qg, ps_y, ps_l))

                while pending_norm:
                    emit_norm()

            # ---- output projection: outT[oc, t] = wp^T-slice @ yT ----
            wp_sb = wpool.tile([P, 4, T], F32R, tag="w")
            nc.sync.dma_start(wp_sb[:], wpr)
            ytr = yt_dram.rearrange("(hc p) t -> p hc t", p=P)
            yt_sb = [xtp.tile([P, T], F32R, tag="xt", name=f"yt{i}") for i in range(4)]
            for hc in range(4):
                nc.sync.dma_start(yt_sb[hc][:], ytr[:, hc, :])
            for oc in range(16):
                for tg in range(TG):
                    ps_o = mmp.tile([P, QG], F32, tag="mm")
                    for hc in range(4):
                        nc.tensor.matmul(
                            ps_o[:],
                            r(wp_sb[:, hc, oc * 128:(oc + 1) * 128]),
                            r(yt_sb[hc][:, tg * QG:(tg + 1) * QG]),
                            start=(hc == 0),
                            stop=(hc == 3),
                        )
                    stage = work.tile([P, QG], F32, tag="tmp")
                    nc.scalar.activation(stage[:], ps_o[:], AF.Copy)
                    nc.sync.dma_start(
                        outT[oc * 128:(oc + 1) * 128, tg * QG:(tg + 1) * QG],
                        stage[:],
                    )

    nc.finalize()
    return nc


def _host_inputs(x, freqs_cis, w_attn, w_proj):
    """Build the 8 per-core input maps."""
    x = np.asarray(x, dtype=np.float32)
    freqs_cis = np.asarray(freqs_cis, dtype=np.float32)
    w_attn = np.asarray(w_attn, dtype=np.float32)
    w_proj = np.asarray(w_proj, dtype=np.float32)

    B = x.shape[0]
    perm = np.concatenate([np.arange(0, HSIZE, 2), np.arange(1, HSIZE, 2)])

    cos = np.ascontiguousarray(freqs_cis[:, :, 0].T)  # [64, T]
    sin = np.ascontiguousarray(freqs_cis[:, :, 1].T)
    c1 = np.concatenate([cos, cos], axis=0)           # [128, T]
    c2 = np.concatenate([-sin, sin], axis=0)

    kk = np.arange(P)[:, None]
    ccol = np.arange(QG)[None, :]
    mk = np.stack(
        [(ccol >= s * 128 + kk).astype(np.float32) for s in range(4)], axis=0
    )  # [4,128,512]

    ones_col = np.ones((P, 1), np.float32)
    ones_row = np.ones((1, P), np.float32)

    xT = [np.ascontiguousarray(x[b].T) for b in range(B)]

    in_maps = []
    for core in range(N_CORES):
        b, g = core // 4, core % 4
        blocks = []
        for pairp in range(2):
            for off in (0, C, 2 * C):  # q, k, v origins in w_attn
                for hh in range(2):
                    hglob = 4 * g + 2 * pairp + hh
                    cols = w_attn[:, off + hglob * HSIZE: off + (hglob + 1) * HSIZE]
                    if off != 2 * C:  # permute q and k, not v
                        cols = cols[:, perm]
                    blocks.append(cols)
        wcore = np.ascontiguousarray(np.concatenate(blocks, axis=1))  # [C, 1536]
        wpcore = np.ascontiguousarray(w_proj[g * 512:(g + 1) * 512, :])
        in_maps.append(
            {
                "xT": xT[b],
                "w": wcore,
                "wp": wpcore,
                "c1": c1,
                "c2": c2,
                "mk": mk,
                "ones_col": ones_col,
                "ones_row": ones_row,
            }
        )
    return in_maps


_LAST_RESULT = {}


def kernel(x, freqs_cis, w_attn, w_proj):
    if _TRACE:
        _install_ntff_hook()
    in_maps = _host_inputs(x, freqs_cis, w_attn, w_proj)
    nc = build_nc()
    res = run_bass_kernel_spmd(
        nc, in_maps, core_ids=list(range(N_CORES)), trace=_TRACE
    )
    _LAST_RESULT["res"] = res

    B = x.shape[0]
    out = np.zeros((B, T, C), dtype=np.float32)
    for core in range(N_CORES):
        b = core // 4
        out[b] += res.results[core]["outT"].T
    return out


# revision 10
# speedup vs baseline: 1.2151x; 1.0797x over previous
"""Causal self-attention (dense transformer block) for 8 Trainium2 NeuronCores.

Sharding: DP over batch (2) x TP over heads (4 groups of 4 heads) = 8 cores.
Per core: column-parallel QKV projection (4 heads), RoPE, causal
flash-attention (no-max-subtraction softmax with constant bias), row-parallel
output projection producing a partial [oc, t] result; host sums the 4 TP
partials per batch and transposes back.

Device matmuls run as float32r (full-rate fp32 storage, ~tf32 accuracy).

Self-contained: hardcodes shapes, builds/compiles/runs the Bass kernel via
run_bass_kernel_spmd on cores 0-7.
"""

import os
import sys
import types

sys.path.insert(0, "/opt/trn_rl_repo")

import numpy as np

import concourse.bass as bass
import concourse.mybir as mybir
import concourse.tile as tile
from concourse import bacc
from concourse.bass_utils import run_bass_kernel_spmd
from concourse.vector_clock import ScopedClock, VectorClock

F32 = mybir.dt.float32
F32R = mybir.dt.float32r
AF = mybir.ActivationFunctionType
ALU = mybir.AluOpType

P = 128
T = 2048
C = 2048
NH = 16          # total heads
HPC = 4          # heads per core
HSIZE = 128
N_CORES = 8
TG = 4           # t-groups of 512
QG = 512
EXP_BIAS = -10.0
SCALE = 1.0 / float(np.sqrt(HSIZE))

_TRACE = os.environ.get("BASS_KERNEL_TRACE", "0") == "1"


def _patch_tile_drain():
    """walrus in this toolchain allows at most one sync-wait per instruction;
    TileContext's tail drain aggregates the whole global clock onto one Drain.
    Split it: one Drain per pending proc, each with a single wait."""
    if getattr(tile.TileContext, "_drain_patched", False):
        return

    def _drain_and_barrier(self, tick_clock, wait_clock):
        nc = self.nc
        gc = tick_clock.global_clock
        n = len(gc)
        for p in range(n):
            if gc[p] > 0:
                vc = VectorClock([gc[p] if i == p else 0 for i in range(n)])
                di = nc.sync.drain()
                wait_clock.add_sem_waits(di.ins, ScopedClock({None: vc}))
        nc.all_engine_barrier()
        popped = nc._tile_sem_poison_stack.pop()
        assert popped is self._sem_poison
        nc.clear_and_free_semaphores(list(self.sems.allocated().values()))
        nc.all_engine_barrier()

    tile.TileContext._drain_and_barrier = _drain_and_barrier
    tile.TileContext._drain_patched = True


def _install_ntff_hook():
    """Wire the axon NTFF profiling hook this image leaves unwired (the agent
    image's antenv lacks axon_hooks). Only needed when tracing."""
    import antenv

    if getattr(antenv, "axon_hooks", None) is not None:
        return
    mod = types.ModuleType("antenv.axon_hooks")
    mod._hook = None
    mod.set_axon_ntff_profile_hook = lambda h: setattr(mod, "_hook", h)
    mod.get_axon_ntff_profile_hook = lambda: mod._hook
    sys.modules["antenv.axon_hooks"] = mod
    antenv.axon_hooks = mod
    if "/root/.axon_site" not in sys.path:
        sys.path.insert(0, "/root/.axon_site")
    try:
        from trn_agent_boot.trn_boot import _ntff_profile_via_ctypes

        hook = _ntff_profile_via_ctypes("/opt/axon/libaxon_pjrt.so")
        if hook is not None:
            mod.set_axon_ntff_profile_hook(hook)
        import concourse.bass_utils as bu

        bu.upload_artifacts = lambda d: d
    except Exception:
        pass


def build_nc():
    _patch_tile_drain()
    nc = bacc.Bacc(None, target_bir_lowering=False)

    xT = nc.dram_tensor("xT", [C, T], F32R, kind="ExternalInput")
    w = nc.dram_tensor("w", [C, 6 * HSIZE * 2], F32R, kind="ExternalInput")  # [C,1536]
    wp = nc.dram_tensor("wp", [HPC * HSIZE, T], F32R, kind="ExternalInput")  # [512,T]
    c1d = nc.dram_tensor("c1", [P, T], F32R, kind="ExternalInput")
    c2d = nc.dram_tensor("c2", [P, T], F32R, kind="ExternalInput")
    mkd = nc.dram_tensor("mk", [4, P, QG], F32R, kind="ExternalInput")
    onesd = nc.dram_tensor("ones_col", [P, 1], F32R, kind="ExternalInput")
    onesrd = nc.dram_tensor("ones_row", [1, P], F32R, kind="ExternalInput")
    outT = nc.dram_tensor("outT", [T, T], F32, kind="ExternalOutput")  # [oc, t]

    xTr = xT.rearrange("(cc p) t -> p cc t", p=P)      # [128,16,2048]
    wr = w.rearrange("(cc p) j -> p cc j", p=P)        # [128,16,1536]
    wpr = wp.rearrange("(hc p) t -> p hc t", p=P)      # [128,4,2048]
    mkr = mkd.rearrange("s p q -> p s q")              # [128,4,512]

    def r(ap):
        return ap

    with tile.TileContext(nc) as tc, nc.allow_low_precision(
        reason="f32r storage is the intended reduced-precision matmul format"
    ):
        with (
            tc.tile_pool(name="const", bufs=1) as constp,
            tc.tile_pool(name="wpool", bufs=1) as wpool,
            tc.tile_pool(name="xtp", bufs=5) as xtp,
            tc.tile_pool(name="qk", bufs=1) as qkres,
            tc.tile_pool(name="vres", bufs=1) as vresp,
            tc.tile_pool(name="work", bufs=6) as work,
            tc.tile_pool(name="pwork", bufs=6) as pwork,
            tc.tile_pool(name="rp", bufs=2) as rpool,
            tc.tile_pool(name="mm", bufs=4, space="PSUM") as mmp,
            tc.tile_pool(name="yt", bufs=2, space="PSUM") as ytp,
            tc.tile_pool(name="lp", bufs=2, space="PSUM") as lpp,
            tc.tile_pool(name="dram", bufs=1, space="DRAM") as dramp,
        ):
            c1 = constp.tile([P, T], F32R, tag="c1")
            c2 = constp.tile([P, T], F32R, tag="c2")
            mk = constp.tile([P, 4, QG], F32R, tag="mk")
            ones_c = constp.tile([P, 1], F32R, tag="onc")
            ones_r = constp.tile([1, P], F32R, tag="onr")
            nc.sync.dma_start(c1[:], c1d[:])
            nc.sync.dma_start(c2[:], c2d[:])
            nc.sync.dma_start(mk[:], mkr)
            nc.sync.dma_start(ones_c[:], onesd[:])
            ebias = constp.tile([P, 1], F32, tag="ebias")
            nc.gpsimd.memset(ebias[:], EXP_BIAS)
            nc.sync.dma_start(ones_r[:], onesrd[:])

            yt_dram = dramp.tile([HPC * HSIZE, T], F32R)  # [512, 2048] spill

            for pair in range(2):
                w_sb = wpool.tile([P, 16, 768], F32R, tag="w")
                nc.sync.dma_start(w_sb[:], wr[:, :, pair * 768:(pair + 1) * 768])

                q_sb = [qkres.tile([P, T], F32R, tag=f"q{h}", name=f"q{h}") for h in range(2)]
                k_sb = [qkres.tile([P, T], F32R, tag=f"k{h}", name=f"k{h}") for h in range(2)]
                v_sb = vresp.tile([P, 16, 256], F32R, tag="v")

                # ---- QKV projection for this pair ----
                for tg in range(TG):
                    xts = []
                    for ch in range(4):
                        xt = xtp.tile([P, 4, QG], F32R, tag="xt")
                        nc.sync.dma_start(
                            xt[:],
                            xTr[:, ch * 4:(ch + 1) * 4, tg * QG:(tg + 1) * QG],
                        )
                        xts.append(xt)

                    # q/k: 4 j-tiles (q_h0, q_h1, k_h0, k_h1), N=512,
                    # in two passes of 2 concurrent psums to keep mm-pool slack
                    def rope(j, psum):
                        dst = (q_sb if j < 2 else k_sb)[j % 2]
                        dsl = dst[:, tg * QG:(tg + 1) * QG]
                        qraw = work.tile([P, QG], F32R, tag="tmp", name="qraw")
                        nc.scalar.activation(qraw[:], psum[:], AF.Copy)
                        qsw = work.tile([P, QG], F32R, tag="tmp", name="qsw")
                        nc.gpsimd.dma_start(qsw[0:64, :], qraw[64:128, :])
                        nc.gpsimd.dma_start(qsw[64:128, :], qraw[0:64, :])
                        t2 = work.tile([P, QG], F32R, tag="tmp", name="t2")
                        c1s = c1[:, tg * QG:(tg + 1) * QG]
                        c2s = c2[:, tg * QG:(tg + 1) * QG]
                        nc.vector.tensor_mul(dsl, qraw[:], c1s)
                        nc.vector.tensor_mul(t2[:], qsw[:], c2s)
                        nc.vector.tensor_add(dsl, dsl, t2[:])

                    for jp in range(2):
                        psq = [mmp.tile([P, QG], F32, tag="mm", name=f"psq{j}")
                               for j in range(2)]
                        for cc in range(16):
                            xt = xts[cc // 4][:, cc % 4, :]
                            for j in range(2):
                                nc.tensor.matmul(
                                    psq[j][:],
                                    r(w_sb[:, cc, (jp * 2 + j) * 128:(jp * 2 + j + 1) * 128]),
                                    r(xt),
                                    start=(cc == 0),
                                    stop=(cc == 15),
                                )
                        rope(jp * 2 + 0, psq[0])
                        rope(jp * 2 + 1, psq[1])
                    # v: 4 t-tiles in this tg, N=256 (both heads' v); yt pool is idle here
                    for tt in range(4):
                        psv = ytp.tile([P, 256], F32, tag="yt", name="psv")
                        for cc in range(16):
                            nc.tensor.matmul(
                                psv[:],
                                r(xts[cc // 4][:, cc % 4, tt * 128:(tt + 1) * 128]),
                                r(w_sb[:, cc, 512:768]),
                                start=(cc == 0),
                                stop=(cc == 15),
                            )
                        nc.vector.tensor_copy(v_sb[:, tg * 4 + tt, :], psv[:])

                # ---- attention for the pair's 2 heads ----
                pending_norm = []

                def emit_norm():
                    hg_, qg_, ps_y_, ps_l_ = pending_norm.pop(0)
                    r_f32 = rpool.tile([1, QG], F32, tag="rf", name="r_f32")
                    nc.vector.reciprocal_approx_fast(r_f32[:], ps_l_[:])
                    r_sb = rpool.tile([1, QG], F32R, tag="r", name="r_sb")
                    nc.scalar.activation(r_sb[:], r_f32[:], AF.Copy)
                    ps_r = mmp.tile([P, QG], F32, tag="mm", name="ps_r")
                    nc.tensor.matmul(
                        ps_r[:], r(ones_r[:]), r(r_sb[:]), start=True, stop=True
                    )
                    r128 = pwork.tile([P, QG], F32, tag="p", name="r128")
                    nc.scalar.activation(r128[:], ps_r[:], AF.Copy)
                    stage = work.tile([P, QG], F32R, tag="tmp", name="stage")
                    nc.vector.tensor_mul(stage[:], ps_y_[:], r128[:])
                    nc.sync.dma_start(
                        yt_dram[hg_ * 128:(hg_ + 1) * 128, qg_ * QG:(qg_ + 1) * QG],
                        stage[:],
                    )

                for h in range(2):
                    hg = pair * 2 + h
                    for qg in range(TG):
                        n_kt = 4 * qg + 4
                        LA = 2
                        ps_y = ytp.tile([P, QG], F32, tag="yt")
                        ps_l = lpp.tile([1, QG], F32, tag="l")
                        p_tiles = {}

                        def emit_s(kt):
                            ps_s = mmp.tile([P, QG], F32, tag="mm", name="ps_s")
                            nc.tensor.matmul(
                                ps_s[:],
                                r(k_sb[h][:, kt * 128:(kt + 1) * 128]),
                                r(q_sb[h][:, qg * QG:(qg + 1) * QG]),
                                start=True,
                                stop=True,
                            )
                            p_sb = pwork.tile([P, QG], F32R, tag="p", name="p_sb")
                            nc.scalar.activation(
                                p_sb[:], ps_s[:], AF.Exp, bias=ebias[:], scale=SCALE
                            )
                            if kt >= 4 * qg:
                                s = kt - 4 * qg
                                nc.vector.tensor_mul(p_sb[:], p_sb[:], mk[:, s, :])
                            p_tiles[kt] = p_sb

                        def emit_av(kt):
                            p_sb = p_tiles.pop(kt)
                            nc.tensor.matmul(
                                ps_y[:],
                                r(v_sb[:, kt, h * 128:(h + 1) * 128]),
                                r(p_sb[:]),
                                start=(kt == 0),
                                stop=(kt == n_kt - 1),
                            )
                            nc.tensor.matmul(
                                ps_l[:],
                                r(ones_c[:]),
                                r(p_sb[:]),
                                start=(kt == 0),
                                stop=(kt == n_kt - 1),
                            )

                        for kt in range(n_kt + LA):
                            if kt < n_kt:
                                emit_s(kt)
                            if kt == 3 and pending_norm:
                                emit_norm()
                            if kt >= LA:
                                emit_av(kt - LA)
                        pending_norm.append((hg, qg, ps_y, ps_l))

                while pending_norm:
                    emit_norm()

            # ---- output projection: outT[oc, t] = wp^T-slice @ yT ----
            wp_sb = wpool.tile([P, 4, T], F32R, tag="w")
            nc.sync.dma_start(wp_sb[:], wpr)
            ytr = yt_dram.rearrange("(hc p) t -> p hc t", p=P)
            yt_sb = [xtp.tile([P, T], F32R, tag="xt", name=f"yt{i}") for i in range(4)]
            for hc in range(4):
                nc.sync.dma_start(yt_sb[hc][:], ytr[:, hc, :])
            for oc in range(16):
                for tg in range(TG):
                    ps_o = mmp.tile([P, QG], F32, tag="mm")
                    for hc in range(4):
                        nc.tensor.matmul(
                            ps_o[:],
                            r(wp_sb[:, hc, oc * 128:(oc + 1) * 128]),
                            r(yt_sb[hc][:, tg * QG:(tg + 1) * QG]),
                            start=(hc == 0),
                            stop=(hc == 3),
                        )
                    stage = work.tile([P, QG], F32, tag="tmp")
                    nc.scalar.activation(stage[:], ps_o[:], AF.Copy)
                    nc.sync.dma_start(
                        outT[oc * 128:(oc + 1) * 128, tg * QG:(tg + 1) * QG],
                        stage[:],
                    )

    nc.finalize()
    return nc


def _host_inputs(x, freqs_cis, w_attn, w_proj):
    """Build the 8 per-core input maps."""
    x = np.asarray(x, dtype=np.float32)
    freqs_cis = np.asarray(freqs_cis, dtype=np.float32)
    w_attn = np.asarray(w_attn, dtype=np.float32)
    w_proj = np.asarray(w_proj, dtype=np.float32)

    B = x.shape[0]
    perm = np.concatenate([np.arange(0, HSIZE, 2), np.arange(1, HSIZE, 2)])

    cos = np.ascontiguousarray(freqs_cis[:, :, 0].T)  # [64, T]
    sin = np.ascontiguousarray(freqs_cis[:, :, 1].T)
    c1 = np.concatenate([cos, cos], axis=0)           # [128, T]
    c2 = np.concatenate([-sin, sin], axis=0)

    kk = np.arange(P)[:, None]
    ccol = np.arange(QG)[None, :]
    mk = np.stack(
        [(ccol >= s * 128 + kk).astype(np.float32) for s in range(4)], axis=0
    )  # [4,128,512]

    ones_col = np.ones((P, 1), np.float32)
    ones_row = np.ones((1, P), np.float32)

    xT = [np.ascontiguousarray(x[b].T) for b in range(B)]

    in_maps = []
    for core in range(N_CORES):
        b, g = core // 4, core % 4
        blocks = []
        for pairp in range(2):
            for off in (0, C, 2 * C):  # q, k, v origins in w_attn
                for hh in range(2):
                    hglob = 4 * g + 2 * pairp + hh
                    cols = w_attn[:, off + hglob * HSIZE: off + (hglob + 1) * HSIZE]
                    if off != 2 * C:  # permute q and k, not v
                        cols = cols[:, perm]
                    blocks.append(cols)
        wcore = np.ascontiguousarray(np.concatenate(blocks, axis=1))  # [C, 1536]
        wpcore = np.ascontiguousarray(w_proj[g * 512:(g + 1) * 512, :])
        in_maps.append(
            {
                "xT": xT[b],
                "w": wcore,
                "wp": wpcore,
                "c1": c1,
                "c2": c2,
                "mk": mk,
                "ones_col": ones_col,
                "ones_row": ones_row,
            }
        )
    return in_maps


_LAST_RESULT = {}


def kernel(x, freqs_cis, w_attn, w_proj):
    if _TRACE:
        _install_ntff_hook()
    in_maps = _host_inputs(x, freqs_cis, w_attn, w_proj)
    nc = build_nc()
    res = run_bass_kernel_spmd(
        nc, in_maps, core_ids=list(range(N_CORES)), trace=_TRACE
    )
    _LAST_RESULT["res"] = res

    B = x.shape[0]
    out = np.zeros((B, T, C), dtype=np.float32)
    for core in range(N_CORES):
        b = core // 4
        out[b] += res.results[core]["outT"].T
    return out


# revision 13
# speedup vs baseline: 1.2378x; 1.0187x over previous
"""Causal self-attention (dense transformer block) for 8 Trainium2 NeuronCores.

Sharding: DP over batch (2) x TP over heads (4 groups of 4 heads) = 8 cores.
Per core: column-parallel QKV projection (4 heads), RoPE, causal
flash-attention (no-max-subtraction softmax with constant bias), row-parallel
output projection producing a partial [oc, t] result; host sums the 4 TP
partials per batch and transposes back.

Device matmuls run as float32r (full-rate fp32 storage, ~tf32 accuracy).

Self-contained: hardcodes shapes, builds/compiles/runs the Bass kernel via
run_bass_kernel_spmd on cores 0-7.
"""

import os
import sys
import types

sys.path.insert(0, "/opt/trn_rl_repo")

import numpy as np

import concourse.bass as bass
import concourse.mybir as mybir
import concourse.tile as tile
from concourse import bacc
from concourse.bass_utils import run_bass_kernel_spmd
from concourse.vector_clock import ScopedClock, VectorClock

F32 = mybir.dt.float32
F32R = mybir.dt.float32r
AF = mybir.ActivationFunctionType
ALU = mybir.AluOpType

P = 128
T = 2048
C = 2048
NH = 16          # total heads
HPC = 4          # heads per core
HSIZE = 128
N_CORES = 8
TG = 4           # t-groups of 512
QG = 512
EXP_BIAS = -10.0
SCALE = 1.0 / float(np.sqrt(HSIZE))

_TRACE = os.environ.get("BASS_KERNEL_TRACE", "0") == "1"


def _patch_tile_drain():
    """walrus in this toolchain allows at most one sync-wait per instruction;
    TileContext's tail drain aggregates the whole global clock onto one Drain.
    Split it: one Drain per pending proc, each with a single wait."""
    if getattr(tile.TileContext, "_drain_patched", False):
        return

    def _drain_and_barrier(self, tick_clock, wait_clock):
        nc = self.nc
        gc = tick_clock.global_clock
        n = len(gc)
        for p in range(n):
            if gc[p] > 0:
                vc = VectorClock([gc[p] if i == p else 0 for i in range(n)])
                di = nc.sync.drain()
                wait_clock.add_sem_waits(di.ins, ScopedClock({None: vc}))
        nc.all_engine_barrier()
        popped = nc._tile_sem_poison_stack.pop()
        assert popped is self._sem_poison
        nc.clear_and_free_semaphores(list(self.sems.allocated().values()))
        nc.all_engine_barrier()

    tile.TileContext._drain_and_barrier = _drain_and_barrier
    tile.TileContext._drain_patched = True


def _install_ntff_hook():
    """Wire the axon NTFF profiling hook this image leaves unwired (the agent
    image's antenv lacks axon_hooks). Only needed when tracing."""
    import antenv

    if getattr(antenv, "axon_hooks", None) is not None:
        return
    mod = types.ModuleType("antenv.axon_hooks")
    mod._hook = None
    mod.set_axon_ntff_profile_hook = lambda h: setattr(mod, "_hook", h)
    mod.get_axon_ntff_profile_hook = lambda: mod._hook
    sys.modules["antenv.axon_hooks"] = mod
    antenv.axon_hooks = mod
    if "/root/.axon_site" not in sys.path:
        sys.path.insert(0, "/root/.axon_site")
    try:
        from trn_agent_boot.trn_boot import _ntff_profile_via_ctypes

        hook = _ntff_profile_via_ctypes("/opt/axon/libaxon_pjrt.so")
        if hook is not None:
            mod.set_axon_ntff_profile_hook(hook)
        import concourse.bass_utils as bu

        bu.upload_artifacts = lambda d: d
    except Exception:
        pass


def build_nc():
    _patch_tile_drain()
    nc = bacc.Bacc(None, target_bir_lowering=False)

    xT = nc.dram_tensor("xT", [C, T], F32R, kind="ExternalInput")
    w = nc.dram_tensor("w", [C, 6 * HSIZE * 2], F32R, kind="ExternalInput")  # [C,1536]
    wp = nc.dram_tensor("wp", [HPC * HSIZE, T], F32R, kind="ExternalInput")  # [512,T]
    c1d = nc.dram_tensor("c1", [P, T], F32R, kind="ExternalInput")
    c2d = nc.dram_tensor("c2", [P, T], F32R, kind="ExternalInput")
    mkd = nc.dram_tensor("mk", [4, P, QG], F32R, kind="ExternalInput")
    onesd = nc.dram_tensor("ones_col", [P, 1], F32R, kind="ExternalInput")
    onesrd = nc.dram_tensor("ones_row", [1, P], F32R, kind="ExternalInput")
    swpd = nc.dram_tensor("swp", [P, P], F32R, kind="ExternalInput")
    outT = nc.dram_tensor("outT", [T, T], F32, kind="ExternalOutput")  # [oc, t]

    xTr = xT.rearrange("(cc p) t -> p cc t", p=P)      # [128,16,2048]
    wr = w.rearrange("(cc p) j -> p cc j", p=P)        # [128,16,1536]
    wpr = wp.rearrange("(hc p) t -> p hc t", p=P)      # [128,4,2048]
    mkr = mkd.rearrange("s p q -> p s q")              # [128,4,512]

    def r(ap):
        return ap

    with tile.TileContext(nc) as tc, nc.allow_low_precision(
        reason="f32r storage is the intended reduced-precision matmul format"
    ):
        with (
            tc.tile_pool(name="const", bufs=1) as constp,
            tc.tile_pool(name="wpool", bufs=1) as wpool,
            tc.tile_pool(name="xtp", bufs=5) as xtp,
            tc.tile_pool(name="qk", bufs=1) as qkres,
            tc.tile_pool(name="vres", bufs=1) as vresp,
            tc.tile_pool(name="work", bufs=6) as work,
            tc.tile_pool(name="pwork", bufs=6) as pwork,
            tc.tile_pool(name="rp", bufs=2) as rpool,
            tc.tile_pool(name="mm", bufs=4, space="PSUM") as mmp,
            tc.tile_pool(name="yt", bufs=2, space="PSUM") as ytp,
            tc.tile_pool(name="lp", bufs=2, space="PSUM") as lpp,
            tc.tile_pool(name="dram", bufs=1, space="DRAM") as dramp,
        ):
            c1 = constp.tile([P, T], F32R, tag="c1")
            c2 = constp.tile([P, T], F32R, tag="c2")
            mk = constp.tile([P, 4, QG], F32R, tag="mk")
            ones_c = constp.tile([P, 1], F32R, tag="onc")
            ones_r = constp.tile([1, P], F32R, tag="onr")
            swp = constp.tile([P, P], F32R, tag="swp")
            nc.sync.dma_start(c1[:], c1d[:])
            nc.sync.dma_start(c2[:], c2d[:])
            nc.sync.dma_start(mk[:], mkr)
            nc.sync.dma_start(ones_c[:], onesd[:])
            ebias = constp.tile([P, 1], F32, tag="ebias")
            nc.gpsimd.memset(ebias[:], EXP_BIAS)
            nc.sync.dma_start(ones_r[:], onesrd[:])
            nc.sync.dma_start(swp[:], swpd[:])

            yt_dram = dramp.tile([HPC * HSIZE, T], F32R)  # [512, 2048] spill
            ytr = yt_dram.rearrange("(hc p) t -> p hc t", p=P)
            yt_sb = []

            for pair in range(2):
                w_sb = wpool.tile([P, 16, 768], F32R, tag="w")
                nc.sync.dma_start(w_sb[:], wr[:, :, pair * 768:(pair + 1) * 768])

                q_sb = [qkres.tile([P, T], F32R, tag=f"q{h}", name=f"q{h}") for h in range(2)]
                k_sb = [qkres.tile([P, T], F32R, tag=f"k{h}", name=f"k{h}") for h in range(2)]
                v_sb = vresp.tile([P, 16, 256], F32R, tag="v")

                # ---- QKV projection for this pair ----
                for tg in range(TG):
                    xts = []
                    for ch in range(4):
                        xt = xtp.tile([P, 4, QG], F32R, tag="xt")
                        nc.sync.dma_start(
                            xt[:],
                            xTr[:, ch * 4:(ch + 1) * 4, tg * QG:(tg + 1) * QG],
                        )
                        xts.append(xt)

                    # q/k: 4 j-tiles (q_h0, q_h1, k_h0, k_h1), N=512,
                    # in two passes of 2 concurrent psums to keep mm-pool slack
                    def rope(j, psum):
                        dst = (q_sb if j < 2 else k_sb)[j % 2]
                        dsl = dst[:, tg * QG:(tg + 1) * QG]
                        qraw = work.tile([P, QG], F32R, tag="tmp", name="qraw")
                        nc.scalar.activation(qraw[:], psum[:], AF.Copy)
                        ps_sw = mmp.tile([P, QG], F32, tag="mm", name="ps_sw")
                        nc.tensor.matmul(
                            ps_sw[:], swp[:], qraw[:], start=True, stop=True
                        )
                        t2 = work.tile([P, QG], F32R, tag="tmp", name="t2")
                        c1s = c1[:, tg * QG:(tg + 1) * QG]
                        c2s = c2[:, tg * QG:(tg + 1) * QG]
                        nc.vector.tensor_mul(dsl, qraw[:], c1s)
                        nc.vector.tensor_mul(t2[:], ps_sw[:], c2s)
                        nc.vector.tensor_add(dsl, dsl, t2[:])

                    for jp in range(2):
                        psq = [mmp.tile([P, QG], F32, tag="mm", name=f"psq{j}")
                               for j in range(2)]
                        for cc in range(16):
                            xt = xts[cc // 4][:, cc % 4, :]
                            for j in range(2):
                                nc.tensor.matmul(
                                    psq[j][:],
                                    r(w_sb[:, cc, (jp * 2 + j) * 128:(jp * 2 + j + 1) * 128]),
                                    r(xt),
                                    start=(cc == 0),
                                    stop=(cc == 15),
                                )
                        rope(jp * 2 + 0, psq[0])
                        rope(jp * 2 + 1, psq[1])
                    # v: 4 t-tiles in this tg, N=256 (both heads' v); yt pool is idle here
                    for tt in range(4):
                        psv = ytp.tile([P, 256], F32, tag="yt", name="psv")
                        for cc in range(16):
                            nc.tensor.matmul(
                                psv[:],
                                r(xts[cc // 4][:, cc % 4, tt * 128:(tt + 1) * 128]),
                                r(w_sb[:, cc, 512:768]),
                                start=(cc == 0),
                                stop=(cc == 15),
                            )
                        nc.vector.tensor_copy(v_sb[:, tg * 4 + tt, :], psv[:])

                # ---- attention for the pair's 2 heads ----
                if pair == 1:
                    # prefetch proj weights + pair0's yT during pair1 attention
                    wp_sb = wpool.tile([P, 4, T], F32R, tag="w", name="wp_sb")
                    for ocq in range(4):
                        nc.sync.dma_start(
                            wp_sb[:, :, ocq * 512:(ocq + 1) * 512],
                            wpr[:, :, ocq * 512:(ocq + 1) * 512],
                        )
                    yt_sb.extend(
                        xtp.tile([P, T], F32R, tag="xt", name=f"yt{i}")
                        for i in range(4)
                    )
                    for hc in range(2):
                        nc.sync.dma_start(yt_sb[hc][:], ytr[:, hc, :])

                pending_norm = []

                def emit_norm():
                    hg_, qg_, ps_y_, ps_l_ = pending_norm.pop(0)
                    r_f32 = rpool.tile([1, QG], F32, tag="rf", name="r_f32")
                    nc.vector.reciprocal_approx_fast(r_f32[:], ps_l_[:])
                    r_sb = rpool.tile([1, QG], F32R, tag="r", name="r_sb")
                    nc.scalar.activation(r_sb[:], r_f32[:], AF.Copy)
                    ps_r = mmp.tile([P, QG], F32, tag="mm", name="ps_r")
                    nc.tensor.matmul(
                        ps_r[:], r(ones_r[:]), r(r_sb[:]), start=True, stop=True
                    )
                    r128 = pwork.tile([P, QG], F32, tag="p", name="r128")
                    nc.scalar.activation(r128[:], ps_r[:], AF.Copy)
                    stage = work.tile([P, QG], F32R, tag="tmp", name="stage")
                    nc.vector.tensor_mul(stage[:], ps_y_[:], r128[:])
                    nc.sync.dma_start(
                        yt_dram[hg_ * 128:(hg_ + 1) * 128, qg_ * QG:(qg_ + 1) * QG],
                        stage[:],
                    )

                for h in range(2):
                    hg = pair * 2 + h
                    for qg in range(TG):
                        n_kt = 4 * qg + 4
                        LA = 2
                        ps_y = ytp.tile([P, QG], F32, tag="yt")
                        ps_l = lpp.tile([1, QG], F32, tag="l")
                        p_tiles = {}

                        def emit_s(kt):
                            ps_s = mmp.tile([P, QG], F32, tag="mm", name="ps_s")
                            nc.tensor.matmul(
                                ps_s[:],
                                r(k_sb[h][:, kt * 128:(kt + 1) * 128]),
                                r(q_sb[h][:, qg * QG:(qg + 1) * QG]),
                                start=True,
                                stop=True,
                            )
                            p_sb = pwork.tile([P, QG], F32R, tag="p", name="p_sb")
                            nc.scalar.activation(
                                p_sb[:], ps_s[:], AF.Exp, bias=ebias[:], scale=SCALE
                            )
                            if kt >= 4 * qg:
                                s = kt - 4 * qg
                                nc.vector.tensor_mul(p_sb[:], p_sb[:], mk[:, s, :])
                            p_tiles[kt] = p_sb

                        def emit_av(kt):
                            p_sb = p_tiles.pop(kt)
                            nc.tensor.matmul(
                                ps_y[:],
                                r(v_sb[:, kt, h * 128:(h + 1) * 128]),
                                r(p_sb[:]),
                                start=(kt == 0),
                                stop=(kt == n_kt - 1),
                            )
                            nc.tensor.matmul(
                                ps_l[:],
                                r(ones_c[:]),
                                r(p_sb[:]),
                                start=(kt == 0),
                                stop=(kt == n_kt - 1),
                            )

                        for kt in range(n_kt + LA):
                            if kt < n_kt:
                                emit_s(kt)
                            if kt == 3 and pending_norm:
                                emit_norm()
                            if kt >= LA:
                                emit_av(kt - LA)
                        pending_norm.append((hg, qg, ps_y, ps_l))

                while pending_norm:
                    emit_norm()

            # ---- output projection: outT[oc, t] = wp^T-slice @ yT ----
            for hc in range(2, 4):
                nc.sync.dma_start(yt_sb[hc][:], ytr[:, hc, :])
            for oc in range(16):
                for tg in range(TG):
                    ps_o = mmp.tile([P, QG], F32, tag="mm")
                    for hc in range(4):
                        nc.tensor.matmul(
                            ps_o[:],
                            r(wp_sb[:, hc, oc * 128:(oc + 1) * 128]),
                            r(yt_sb[hc][:, tg * QG:(tg + 1) * QG]),
                            start=(hc == 0),
                            stop=(hc == 3),
                        )
                    stage = work.tile([P, QG], F32, tag="tmp")
                    nc.scalar.activation(stage[:], ps_o[:], AF.Copy)
                    nc.sync.dma_start(
                        outT[oc * 128:(oc + 1) * 128, tg * QG:(tg + 1) * QG],
                        stage[:],
                    )

    nc.finalize()
    return nc


def _host_inputs(x, freqs_cis, w_attn, w_proj):
    """Build the 8 per-core input maps."""
    x = np.asarray(x, dtype=np.float32)
    freqs_cis = np.asarray(freqs_cis, dtype=np.float32)
    w_attn = np.asarray(w_attn, dtype=np.float32)
    w_proj = np.asarray(w_proj, dtype=np.float32)

    B = x.shape[0]
    perm = np.concatenate([np.arange(0, HSIZE, 2), np.arange(1, HSIZE, 2)])

    cos = np.ascontiguousarray(freqs_cis[:, :, 0].T)  # [64, T]
    sin = np.ascontiguousarray(freqs_cis[:, :, 1].T)
    c1 = np.concatenate([cos, cos], axis=0)           # [128, T]
    c2 = np.concatenate([-sin, sin], axis=0)

    kk = np.arange(P)[:, None]
    ccol = np.arange(QG)[None, :]
    mk = np.stack(
        [(ccol >= s * 128 + kk).astype(np.float32) for s in range(4)], axis=0
    )  # [4,128,512]

    ones_col = np.ones((P, 1), np.float32)
    ones_row = np.ones((1, P), np.float32)
    swp = np.zeros((P, P), np.float32)
    for m in range(P):
        swp[(m + 64) % P, m] = 1.0

    xT = [np.ascontiguousarray(x[b].T) for b in range(B)]

    in_maps = []
    for core in range(N_CORES):
        b, g = core // 4, core % 4
        blocks = []
        for pairp in range(2):
            for off in (0, C, 2 * C):  # q, k, v origins in w_attn
                for hh in range(2):
                    hglob = 4 * g + 2 * pairp + hh
                    cols = w_attn[:, off + hglob * HSIZE: off + (hglob + 1) * HSIZE]
                    if off != 2 * C:  # permute q and k, not v
                        cols = cols[:, perm]
                    blocks.append(cols)
        wcore = np.ascontiguousarray(np.concatenate(blocks, axis=1))  # [C, 1536]
        wpcore = np.ascontiguousarray(w_proj[g * 512:(g + 1) * 512, :])
        in_maps.append(
            {
                "xT": xT[b],
                "w": wcore,
                "wp": wpcore,
                "c1": c1,
                "c2": c2,
                "mk": mk,
                "ones_col": ones_col,
                "ones_row": ones_row,
                "swp": swp,
            }
        )
    return in_maps


_LAST_RESULT = {}


def kernel(x, freqs_cis, w_attn, w_proj):
    if _TRACE:
        _install_ntff_hook()
    in_maps = _host_inputs(x, freqs_cis, w_attn, w_proj)
    nc = build_nc()
    res = run_bass_kernel_spmd(
        nc, in_maps, core_ids=list(range(N_CORES)), trace=_TRACE
    )
    _LAST_RESULT["res"] = res

    B = x.shape[0]
    out = np.zeros((B, T, C), dtype=np.float32)
    for core in range(N_CORES):
        b = core // 4
        out[b] += res.results[core]["outT"].T
    return out


# revision 14
# speedup vs baseline: 1.2400x; 1.0018x over previous
"""Causal self-attention (dense transformer block) for 8 Trainium2 NeuronCores.

Sharding: DP over batch (2) x TP over heads (4 groups of 4 heads) = 8 cores.
Per core: column-parallel QKV projection (4 heads), RoPE, causal
flash-attention (no-max-subtraction softmax with constant bias), row-parallel
output projection producing a partial [oc, t] result; host sums the 4 TP
partials per batch and transposes back.

Device matmuls run as float32r (full-rate fp32 storage, ~tf32 accuracy).

Self-contained: hardcodes shapes, builds/compiles/runs the Bass kernel via
run_bass_kernel_spmd on cores 0-7.
"""

import os
import sys
import types

sys.path.insert(0, "/opt/trn_rl_repo")

import numpy as np

import concourse.bass as bass
import concourse.mybir as mybir
import concourse.tile as tile
from concourse import bacc
from concourse.bass_utils import run_bass_kernel_spmd
from concourse.vector_clock import ScopedClock, VectorClock

F32 = mybir.dt.float32
F32R = mybir.dt.float32r
AF = mybir.ActivationFunctionType
ALU = mybir.AluOpType

P = 128
T = 2048
C = 2048
NH = 16          # total heads
HPC = 4          # heads per core
HSIZE = 128
N_CORES = 8
TG = 4           # t-groups of 512
QG = 512
EXP_BIAS = -10.0
SCALE = 1.0 / float(np.sqrt(HSIZE))

_TRACE = os.environ.get("BASS_KERNEL_TRACE", "0") == "1"


def _patch_tile_drain():
    """walrus in this toolchain allows at most one sync-wait per instruction;
    TileContext's tail drain aggregates the whole global clock onto one Drain.
    Split it: one Drain per pending proc, each with a single wait."""
    if getattr(tile.TileContext, "_drain_patched", False):
        return

    def _drain_and_barrier(self, tick_clock, wait_clock):
        nc = self.nc
        gc = tick_clock.global_clock
        n = len(gc)
        for p in range(n):
            if gc[p] > 0:
                vc = VectorClock([gc[p] if i == p else 0 for i in range(n)])
                di = nc.sync.drain()
                wait_clock.add_sem_waits(di.ins, ScopedClock({None: vc}))
        nc.all_engine_barrier()
        popped = nc._tile_sem_poison_stack.pop()
        assert popped is self._sem_poison
        nc.clear_and_free_semaphores(list(self.sems.allocated().values()))
        nc.all_engine_barrier()

    tile.TileContext._drain_and_barrier = _drain_and_barrier
    tile.TileContext._drain_patched = True


def _install_ntff_hook():
    """Wire the axon NTFF profiling hook this image leaves unwired (the agent
    image's antenv lacks axon_hooks). Only needed when tracing."""
    import antenv

    if getattr(antenv, "axon_hooks", None) is not None:
        return
    mod = types.ModuleType("antenv.axon_hooks")
    mod._hook = None
    mod.set_axon_ntff_profile_hook = lambda h: setattr(mod, "_hook", h)
    mod.get_axon_ntff_profile_hook = lambda: mod._hook
    sys.modules["antenv.axon_hooks"] = mod
    antenv.axon_hooks = mod
    if "/root/.axon_site" not in sys.path:
        sys.path.insert(0, "/root/.axon_site")
    try:
        from trn_agent_boot.trn_boot import _ntff_profile_via_ctypes

        hook = _ntff_profile_via_ctypes("/opt/axon/libaxon_pjrt.so")
        if hook is not None:
            mod.set_axon_ntff_profile_hook(hook)
        import concourse.bass_utils as bu

        bu.upload_artifacts = lambda d: d
    except Exception:
        pass


def build_nc():
    _patch_tile_drain()
    nc = bacc.Bacc(None, target_bir_lowering=False)

    xT = nc.dram_tensor("xT", [C, T], F32R, kind="ExternalInput")
    w = nc.dram_tensor("w", [C, 6 * HSIZE * 2], F32R, kind="ExternalInput")  # [C,1536]
    wp = nc.dram_tensor("wp", [HPC * HSIZE, T], F32R, kind="ExternalInput")  # [512,T]
    c1d = nc.dram_tensor("c1", [P, T], F32R, kind="ExternalInput")
    c2d = nc.dram_tensor("c2", [P, T], F32R, kind="ExternalInput")
    mkd = nc.dram_tensor("mk", [4, P, QG], F32R, kind="ExternalInput")
    onesd = nc.dram_tensor("ones_col", [P, 1], F32R, kind="ExternalInput")
    onesrd = nc.dram_tensor("ones_row", [1, P], F32R, kind="ExternalInput")
    swpd = nc.dram_tensor("swp", [P, P], F32R, kind="ExternalInput")
    outT = nc.dram_tensor("outT", [T, T], F32, kind="ExternalOutput")  # [oc, t]

    xTr = xT.rearrange("(cc p) t -> p cc t", p=P)      # [128,16,2048]
    wr = w.rearrange("(cc p) j -> p cc j", p=P)        # [128,16,1536]
    wpr = wp.rearrange("(hc p) t -> p hc t", p=P)      # [128,4,2048]
    mkr = mkd.rearrange("s p q -> p s q")              # [128,4,512]

    def r(ap):
        return ap

    with tile.TileContext(nc) as tc, nc.allow_low_precision(
        reason="f32r storage is the intended reduced-precision matmul format"
    ):
        with (
            tc.tile_pool(name="const", bufs=1) as constp,
            tc.tile_pool(name="wpool", bufs=1) as wpool,
            tc.tile_pool(name="xtp", bufs=5) as xtp,
            tc.tile_pool(name="qk", bufs=1) as qkres,
            tc.tile_pool(name="vres", bufs=1) as vresp,
            tc.tile_pool(name="work", bufs=6) as work,
            tc.tile_pool(name="pwork", bufs=6) as pwork,
            tc.tile_pool(name="rp", bufs=2) as rpool,
            tc.tile_pool(name="mm", bufs=4, space="PSUM") as mmp,
            tc.tile_pool(name="yt", bufs=2, space="PSUM") as ytp,
            tc.tile_pool(name="lp", bufs=2, space="PSUM") as lpp,
            tc.tile_pool(name="dram", bufs=1, space="DRAM") as dramp,
        ):
            c1 = constp.tile([P, T], F32R, tag="c1")
            c2 = constp.tile([P, T], F32R, tag="c2")
            mk = constp.tile([P, 4, QG], F32R, tag="mk")
            ones_c = constp.tile([P, 1], F32R, tag="onc")
            ones_r = constp.tile([1, P], F32R, tag="onr")
            swp = constp.tile([P, P], F32R, tag="swp")
            nc.sync.dma_start(c1[:], c1d[:])
            nc.sync.dma_start(c2[:], c2d[:])
            nc.sync.dma_start(mk[:], mkr)
            nc.sync.dma_start(ones_c[:], onesd[:])
            ebias = constp.tile([P, 1], F32, tag="ebias")
            nc.gpsimd.memset(ebias[:], EXP_BIAS)
            nc.sync.dma_start(ones_r[:], onesrd[:])
            nc.sync.dma_start(swp[:], swpd[:])

            yt_dram = dramp.tile([HPC * HSIZE, T], F32R)  # [512, 2048] spill
            ytr = yt_dram.rearrange("(hc p) t -> p hc t", p=P)
            yt_sb = []

            for pair in range(2):
                w_sb = wpool.tile([P, 16, 768], F32R, tag="w")
                nc.sync.dma_start(w_sb[:], wr[:, :, pair * 768:(pair + 1) * 768])

                q_sb = [qkres.tile([P, T], F32R, tag=f"q{h}", name=f"q{h}") for h in range(2)]
                k_sb = [qkres.tile([P, T], F32R, tag=f"k{h}", name=f"k{h}") for h in range(2)]
                v_sb = vresp.tile([P, 16, 256], F32R, tag="v")

                # ---- QKV projection for this pair ----
                for tg in range(TG):
                    xts = []
                    for ch in range(4):
                        xt = xtp.tile([P, 4, QG], F32R, tag="xt")
                        nc.sync.dma_start(
                            xt[:],
                            xTr[:, ch * 4:(ch + 1) * 4, tg * QG:(tg + 1) * QG],
                        )
                        xts.append(xt)

                    # q/k: 4 j-tiles (q_h0, q_h1, k_h0, k_h1), N=512,
                    # in two passes of 2 concurrent psums to keep mm-pool slack
                    def rope(j, psum):
                        dst = (q_sb if j < 2 else k_sb)[j % 2]
                        dsl = dst[:, tg * QG:(tg + 1) * QG]
                        qraw = work.tile([P, QG], F32R, tag="tmp", name="qraw")
                        nc.scalar.activation(qraw[:], psum[:], AF.Copy)
                        ps_sw = mmp.tile([P, QG], F32, tag="mm", name="ps_sw")
                        nc.tensor.matmul(
                            ps_sw[:], swp[:], qraw[:], start=True, stop=True
                        )
                        t2 = work.tile([P, QG], F32R, tag="tmp", name="t2")
                        c1s = c1[:, tg * QG:(tg + 1) * QG]
                        c2s = c2[:, tg * QG:(tg + 1) * QG]
                        nc.vector.tensor_mul(dsl, qraw[:], c1s)
                        nc.vector.tensor_mul(t2[:], ps_sw[:], c2s)
                        nc.vector.tensor_add(dsl, dsl, t2[:])

                    for jp in range(2):
                        psq = [mmp.tile([P, QG], F32, tag="mm", name=f"psq{j}")
                               for j in range(2)]
                        for cc in range(16):
                            xt = xts[cc // 4][:, cc % 4, :]
                            for j in range(2):
                                nc.tensor.matmul(
                                    psq[j][:],
                                    r(w_sb[:, cc, (jp * 2 + j) * 128:(jp * 2 + j + 1) * 128]),
                                    r(xt),
                                    start=(cc == 0),
                                    stop=(cc == 15),
                                )
                        rope(jp * 2 + 0, psq[0])
                        rope(jp * 2 + 1, psq[1])
                    # v: 4 t-tiles in this tg, N=256 (both heads' v); yt pool is idle here
                    for tt in range(4):
                        psv = ytp.tile([P, 256], F32, tag="yt", name="psv")
                        for cc in range(16):
                            nc.tensor.matmul(
                                psv[:],
                                r(xts[cc // 4][:, cc % 4, tt * 128:(tt + 1) * 128]),
                                r(w_sb[:, cc, 512:768]),
                                start=(cc == 0),
                                stop=(cc == 15),
                            )
                        nc.vector.tensor_copy(v_sb[:, tg * 4 + tt, :], psv[:])

                # ---- attention for the pair's 2 heads ----
                if pair == 1:
                    # prefetch proj weights + pair0's yT during pair1 attention
                    wp_sb = wpool.tile([P, 4, T], F32R, tag="w", name="wp_sb")
                    for ocq in range(4):
                        nc.sync.dma_start(
                            wp_sb[:, :, ocq * 512:(ocq + 1) * 512],
                            wpr[:, :, ocq * 512:(ocq + 1) * 512],
                        )
                    yt_sb.extend(
                        xtp.tile([P, T], F32R, tag="xt", name=f"yt{i}")
                        for i in range(4)
                    )
                    for hc in range(2):
                        nc.sync.dma_start(yt_sb[hc][:], ytr[:, hc, :])

                pending_norm = []

                def emit_norm():
                    hg_, qg_, ps_y_, ps_l_ = pending_norm.pop(0)
                    r_f32 = rpool.tile([1, QG], F32, tag="rf", name="r_f32")
                    nc.vector.reciprocal_approx_fast(r_f32[:], ps_l_[:])
                    r128 = pwork.tile([P, QG], F32, tag="p", name="r128")
                    nc.gpsimd.partition_broadcast(r128[:], r_f32[0:1, :])
                    stage = work.tile([P, QG], F32R, tag="tmp", name="stage")
                    nc.vector.tensor_mul(stage[:], ps_y_[:], r128[:])
                    nc.sync.dma_start(
                        yt_dram[hg_ * 128:(hg_ + 1) * 128, qg_ * QG:(qg_ + 1) * QG],
                        stage[:],
                    )

                for h in range(2):
                    hg = pair * 2 + h
                    for qg in range(TG):
                        n_kt = 4 * qg + 4
                        LA = 3
                        ps_y = ytp.tile([P, QG], F32, tag="yt")
                        ps_l = lpp.tile([1, QG], F32, tag="l")
                        p_tiles = {}

                        def emit_s(kt):
                            ps_s = mmp.tile([P, QG], F32, tag="mm", name="ps_s")
                            nc.tensor.matmul(
                                ps_s[:],
                                r(k_sb[h][:, kt * 128:(kt + 1) * 128]),
                                r(q_sb[h][:, qg * QG:(qg + 1) * QG]),
                                start=True,
                                stop=True,
                            )
                            p_sb = pwork.tile([P, QG], F32R, tag="p", name="p_sb")
                            nc.scalar.activation(
                                p_sb[:], ps_s[:], AF.Exp, bias=ebias[:], scale=SCALE
                            )
                            if kt >= 4 * qg:
                                s = kt - 4 * qg
                                nc.vector.tensor_mul(p_sb[:], p_sb[:], mk[:, s, :])
                            p_tiles[kt] = p_sb

                        def emit_av(kt):
                            p_sb = p_tiles.pop(kt)
                            nc.tensor.matmul(
                                ps_y[:],
                                r(v_sb[:, kt, h * 128:(h + 1) * 128]),
                                r(p_sb[:]),
                                start=(kt == 0),
                                stop=(kt == n_kt - 1),
                            )
                            nc.tensor.matmul(
                                ps_l[:],
                                r(ones_c[:]),
                                r(p_sb[:]),
                                start=(kt == 0),
                                stop=(kt == n_kt - 1),
                            )

                        for kt in range(n_kt + LA):
                            if kt < n_kt:
                                emit_s(kt)
                            if kt == 3 and pending_norm:
                                emit_norm()
                            if kt >= LA:
                                emit_av(kt - LA)
                        pending_norm.append((hg, qg, ps_y, ps_l))

                while pending_norm:
                    emit_norm()

            # ---- output projection: outT[oc, t] = wp^T-slice @ yT ----
            for hc in range(2, 4):
                nc.sync.dma_start(yt_sb[hc][:], ytr[:, hc, :])
            for oc in range(16):
                for tg in range(TG):
                    ps_o = mmp.tile([P, QG], F32, tag="mm")
                    for hc in range(4):
                        nc.tensor.matmul(
                            ps_o[:],
                            r(wp_sb[:, hc, oc * 128:(oc + 1) * 128]),
                            r(yt_sb[hc][:, tg * QG:(tg + 1) * QG]),
                            start=(hc == 0),
                            stop=(hc == 3),
                        )
                    stage = work.tile([P, QG], F32, tag="tmp")
                    nc.scalar.activation(stage[:], ps_o[:], AF.Copy)
                    nc.sync.dma_start(
                        outT[oc * 128:(oc + 1) * 128, tg * QG:(tg + 1) * QG],
                        stage[:],
                    )

    nc.finalize()
    return nc


def _host_inputs(x, freqs_cis, w_attn, w_proj):
    """Build the 8 per-core input maps."""
    x = np.asarray(x, dtype=np.float32)
    freqs_cis = np.asarray(freqs_cis, dtype=np.float32)
    w_attn = np.asarray(w_attn, dtype=np.float32)
    w_proj = np.asarray(w_proj, dtype=np.float32)

    B = x.shape[0]
    perm = np.concatenate([np.arange(0, HSIZE, 2), np.arange(1, HSIZE, 2)])

    cos = np.ascontiguousarray(freqs_cis[:, :, 0].T)  # [64, T]
    sin = np.ascontiguousarray(freqs_cis[:, :, 1].T)
    c1 = np.concatenate([cos, cos], axis=0)           # [128, T]
    c2 = np.concatenate([-sin, sin], axis=0)

    kk = np.arange(P)[:, None]
    ccol = np.arange(QG)[None, :]
    mk = np.stack(
        [(ccol >= s * 128 + kk).astype(np.float32) for s in range(4)], axis=0
    )  # [4,128,512]

    ones_col = np.ones((P, 1), np.float32)
    ones_row = np.ones((1, P), np.float32)
    swp = np.zeros((P, P), np.float32)
    for m in range(P):
        swp[(m + 64) % P, m] = 1.0

    xT = [np.ascontiguousarray(x[b].T) for b in range(B)]

    in_maps = []
    for core in range(N_CORES):
        b, g = core // 4, core % 4
        blocks = []
        for pairp in range(2):
            for off in (0, C, 2 * C):  # q, k, v origins in w_attn
                for hh in range(2):
                    hglob = 4 * g + 2 * pairp + hh
                    cols = w_attn[:, off + hglob * HSIZE: off + (hglob + 1) * HSIZE]
                    if off != 2 * C:  # permute q and k, not v
                        cols = cols[:, perm]
                    blocks.append(cols)
        wcore = np.ascontiguousarray(np.concatenate(blocks, axis=1))  # [C, 1536]
        wpcore = np.ascontiguousarray(w_proj[g * 512:(g + 1) * 512, :])
        in_maps.append(
            {
                "xT": xT[b],
                "w": wcore,
                "wp": wpcore,
                "c1": c1,
                "c2": c2,
                "mk": mk,
                "ones_col": ones_col,
                "ones_row": ones_row,
                "swp": swp,
            }
        )
    return in_maps


_LAST_RESULT = {}


def kernel(x, freqs_cis, w_attn, w_proj):
    if _TRACE:
        _install_ntff_hook()
    in_maps = _host_inputs(x, freqs_cis, w_attn, w_proj)
    nc = build_nc()
    res = run_bass_kernel_spmd(
        nc, in_maps, core_ids=list(range(N_CORES)), trace=_TRACE
    )
    _LAST_RESULT["res"] = res

    B = x.shape[0]
    out = np.zeros((B, T, C), dtype=np.float32)
    for core in range(N_CORES):
        b = core // 4
        out[b] += res.results[core]["outT"].T
    return out


# revision 15
# speedup vs baseline: 1.2713x; 1.0252x over previous
"""Causal self-attention (dense transformer block) for 8 Trainium2 NeuronCores.

Sharding: DP over batch (2) x TP over heads (4 groups of 4 heads) = 8 cores.
Per core: column-parallel QKV projection (4 heads), RoPE, causal
flash-attention (no-max-subtraction softmax with constant bias), row-parallel
output projection producing a partial [oc, t] result; host sums the 4 TP
partials per batch and transposes back.

Device matmuls run as float32r (full-rate fp32 storage, ~tf32 accuracy).

Self-contained: hardcodes shapes, builds/compiles/runs the Bass kernel via
run_bass_kernel_spmd on cores 0-7.
"""

import os
import sys
import types

sys.path.insert(0, "/opt/trn_rl_repo")

import numpy as np

import concourse.bass as bass
import concourse.mybir as mybir
import concourse.tile as tile
from concourse import bacc
from concourse.bass_utils import run_bass_kernel_spmd
from concourse.vector_clock import ScopedClock, VectorClock

F32 = mybir.dt.float32
F32R = mybir.dt.float32r
AF = mybir.ActivationFunctionType
ALU = mybir.AluOpType

P = 128
T = 2048
C = 2048
NH = 16          # total heads
HPC = 4          # heads per core
HSIZE = 128
N_CORES = 8
TG = 4           # t-groups of 512
QG = 512
EXP_BIAS = -10.0
SCALE = 1.0 / float(np.sqrt(HSIZE))

_TRACE = os.environ.get("BASS_KERNEL_TRACE", "0") == "1"


def _patch_tile_drain():
    """walrus in this toolchain allows at most one sync-wait per instruction;
    TileContext's tail drain aggregates the whole global clock onto one Drain.
    Split it: one Drain per pending proc, each with a single wait."""
    if getattr(tile.TileContext, "_drain_patched", False):
        return

    def _drain_and_barrier(self, tick_clock, wait_clock):
        nc = self.nc
        gc = tick_clock.global_clock
        n = len(gc)
        for p in range(n):
            if gc[p] > 0:
                vc = VectorClock([gc[p] if i == p else 0 for i in range(n)])
                di = nc.sync.drain()
                wait_clock.add_sem_waits(di.ins, ScopedClock({None: vc}))
        nc.all_engine_barrier()
        popped = nc._tile_sem_poison_stack.pop()
        assert popped is self._sem_poison
        nc.clear_and_free_semaphores(list(self.sems.allocated().values()))
        nc.all_engine_barrier()

    tile.TileContext._drain_and_barrier = _drain_and_barrier
    tile.TileContext._drain_patched = True


def _install_ntff_hook():
    """Wire the axon NTFF profiling hook this image leaves unwired (the agent
    image's antenv lacks axon_hooks). Only needed when tracing."""
    import antenv

    if getattr(antenv, "axon_hooks", None) is not None:
        return
    mod = types.ModuleType("antenv.axon_hooks")
    mod._hook = None
    mod.set_axon_ntff_profile_hook = lambda h: setattr(mod, "_hook", h)
    mod.get_axon_ntff_profile_hook = lambda: mod._hook
    sys.modules["antenv.axon_hooks"] = mod
    antenv.axon_hooks = mod
    if "/root/.axon_site" not in sys.path:
        sys.path.insert(0, "/root/.axon_site")
    try:
        from trn_agent_boot.trn_boot import _ntff_profile_via_ctypes

        hook = _ntff_profile_via_ctypes("/opt/axon/libaxon_pjrt.so")
        if hook is not None:
            mod.set_axon_ntff_profile_hook(hook)
        import concourse.bass_utils as bu

        bu.upload_artifacts = lambda d: d
    except Exception:
        pass


def build_nc():
    _patch_tile_drain()
    nc = bacc.Bacc(None, target_bir_lowering=False)

    xT = nc.dram_tensor("xT", [C, T], F32R, kind="ExternalInput")
    w = nc.dram_tensor("w", [C, 6 * HSIZE * 2], F32R, kind="ExternalInput")  # [C,1536]
    wp = nc.dram_tensor("wp", [HPC * HSIZE, T], F32R, kind="ExternalInput")  # [512,T]
    c1d = nc.dram_tensor("c1", [P, T], F32R, kind="ExternalInput")
    c2d = nc.dram_tensor("c2", [P, T], F32R, kind="ExternalInput")
    mkd = nc.dram_tensor("mk", [4, P, QG], F32R, kind="ExternalInput")
    onesd = nc.dram_tensor("ones_col", [P, 1], F32R, kind="ExternalInput")
    onesrd = nc.dram_tensor("ones_row", [1, P], F32R, kind="ExternalInput")
    swpd = nc.dram_tensor("swp", [P, P], F32R, kind="ExternalInput")
    outT = nc.dram_tensor("outT", [T, T], F32, kind="ExternalOutput")  # [oc, t]

    xTr = xT.rearrange("(cc p) t -> p cc t", p=P)      # [128,16,2048]
    wr = w.rearrange("(cc p) j -> p cc j", p=P)        # [128,16,1536]
    wpr = wp.rearrange("(hc p) t -> p hc t", p=P)      # [128,4,2048]
    mkr = mkd.rearrange("s p q -> p s q")              # [128,4,512]

    def r(ap):
        return ap

    with tile.TileContext(nc) as tc, nc.allow_low_precision(
        reason="f32r storage is the intended reduced-precision matmul format"
    ):
        with (
            tc.tile_pool(name="const", bufs=1) as constp,
            tc.tile_pool(name="wpool", bufs=1) as wpool,
            tc.tile_pool(name="xtp", bufs=5) as xtp,
            tc.tile_pool(name="qk", bufs=1) as qkres,
            tc.tile_pool(name="vres", bufs=1) as vresp,
            tc.tile_pool(name="work", bufs=8) as work,
            tc.tile_pool(name="pwork", bufs=10) as pwork,
            tc.tile_pool(name="rp", bufs=4) as rpool,
            tc.tile_pool(name="mm", bufs=4, space="PSUM") as mmp,
            tc.tile_pool(name="yt", bufs=2, space="PSUM") as ytp,
            tc.tile_pool(name="lp", bufs=2, space="PSUM") as lpp,
            tc.tile_pool(name="dram", bufs=1, space="DRAM") as dramp,
        ):
            c1 = constp.tile([P, T], F32R, tag="c1")
            c2 = constp.tile([P, T], F32R, tag="c2")
            mk = constp.tile([P, 4, QG], F32R, tag="mk")
            ones_c = constp.tile([P, 1], F32R, tag="onc")
            ones_r = constp.tile([1, P], F32R, tag="onr")
            swp = constp.tile([P, P], F32R, tag="swp")
            nc.sync.dma_start(c1[:], c1d[:])
            nc.sync.dma_start(c2[:], c2d[:])
            nc.sync.dma_start(mk[:], mkr)
            nc.sync.dma_start(ones_c[:], onesd[:])
            ebias = constp.tile([P, 1], F32, tag="ebias")
            nc.gpsimd.memset(ebias[:], EXP_BIAS)
            nc.sync.dma_start(ones_r[:], onesrd[:])
            nc.sync.dma_start(swp[:], swpd[:])

            yt_dram = dramp.tile([HPC * HSIZE, T], F32R)  # [512, 2048] spill
            ytr = yt_dram.rearrange("(hc p) t -> p hc t", p=P)
            yt_sb = []

            for pair in range(2):
                w_sb = wpool.tile([P, 16, 768], F32R, tag="w")
                nc.sync.dma_start(w_sb[:], wr[:, :, pair * 768:(pair + 1) * 768])

                q_sb = [qkres.tile([P, T], F32R, tag=f"q{h}", name=f"q{h}") for h in range(2)]
                k_sb = [qkres.tile([P, T], F32R, tag=f"k{h}", name=f"k{h}") for h in range(2)]
                v_sb = vresp.tile([P, 16, 256], F32R, tag="v")

                # ---- QKV projection for this pair ----
                for tg in range(TG):
                    xts = []
                    for ch in range(4):
                        xt = xtp.tile([P, 4, QG], F32R, tag="xt")
                        nc.sync.dma_start(
                            xt[:],
                            xTr[:, ch * 4:(ch + 1) * 4, tg * QG:(tg + 1) * QG],
                        )
                        xts.append(xt)

                    # q/k: 4 j-tiles (q_h0, q_h1, k_h0, k_h1), N=512,
                    # in two passes of 2 concurrent psums to keep mm-pool slack
                    def rope(j, psum):
                        dst = (q_sb if j < 2 else k_sb)[j % 2]
                        dsl = dst[:, tg * QG:(tg + 1) * QG]
                        qraw = work.tile([P, QG], F32R, tag="tmp", name="qraw")
                        nc.scalar.activation(qraw[:], psum[:], AF.Copy)
                        ps_sw = mmp.tile([P, QG], F32, tag="mm", name="ps_sw")
                        nc.tensor.matmul(
                            ps_sw[:], swp[:], qraw[:], start=True, stop=True
                        )
                        t2 = work.tile([P, QG], F32R, tag="tmp", name="t2")
                        c1s = c1[:, tg * QG:(tg + 1) * QG]
                        c2s = c2[:, tg * QG:(tg + 1) * QG]
                        nc.vector.tensor_mul(dsl, qraw[:], c1s)
                        nc.vector.tensor_mul(t2[:], ps_sw[:], c2s)
                        nc.vector.tensor_add(dsl, dsl, t2[:])

                    for jp in range(2):
                        psq = [mmp.tile([P, QG], F32, tag="mm", name=f"psq{j}")
                               for j in range(2)]
                        for cc in range(16):
                            xt = xts[cc // 4][:, cc % 4, :]
                            for j in range(2):
                                nc.tensor.matmul(
                                    psq[j][:],
                                    r(w_sb[:, cc, (jp * 2 + j) * 128:(jp * 2 + j + 1) * 128]),
                                    r(xt),
                                    start=(cc == 0),
                                    stop=(cc == 15),
                                )
                        rope(jp * 2 + 0, psq[0])
                        rope(jp * 2 + 1, psq[1])
                    # v: 4 t-tiles in this tg, N=256 (both heads' v); yt pool is idle here
                    for tt in range(4):
                        psv = ytp.tile([P, 256], F32, tag="yt", name="psv")
                        for cc in range(16):
                            nc.tensor.matmul(
                                psv[:],
                                r(xts[cc // 4][:, cc % 4, tt * 128:(tt + 1) * 128]),
                                r(w_sb[:, cc, 512:768]),
                                start=(cc == 0),
                                stop=(cc == 15),
                            )
                        nc.vector.tensor_copy(v_sb[:, tg * 4 + tt, :], psv[:])

                # ---- attention for the pair's 2 heads ----
                if pair == 1:
                    # prefetch proj weights + pair0's yT during pair1 attention
                    wp_sb = wpool.tile([P, 4, T], F32R, tag="w", name="wp_sb")
                    for ocq in range(4):
                        nc.sync.dma_start(
                            wp_sb[:, :, ocq * 512:(ocq + 1) * 512],
                            wpr[:, :, ocq * 512:(ocq + 1) * 512],
                        )
                    yt_sb.extend(
                        xtp.tile([P, T], F32R, tag="xt", name=f"yt{i}")
                        for i in range(4)
                    )
                    for hc in range(2):
                        nc.sync.dma_start(yt_sb[hc][:], ytr[:, hc, :])

                pending_norm = []

                def emit_norm():
                    hg_, qg_, ps_y_, ps_l_ = pending_norm.pop(0)
                    r_f32 = rpool.tile([1, QG], F32, tag="rf", name="r_f32")
                    nc.vector.reciprocal_approx_fast(r_f32[:], ps_l_[:])
                    r128 = pwork.tile([P, QG], F32, tag="p", name="r128")
                    nc.gpsimd.partition_broadcast(r128[:], r_f32[0:1, :])
                    stage = work.tile([P, QG], F32R, tag="tmp", name="stage")
                    nc.vector.tensor_mul(stage[:], ps_y_[:], r128[:])
                    nc.sync.dma_start(
                        yt_dram[hg_ * 128:(hg_ + 1) * 128, qg_ * QG:(qg_ + 1) * QG],
                        stage[:],
                    )

                for h in range(2):
                    hg = pair * 2 + h
                    for qg in range(TG):
                        n_kt = 4 * qg + 4
                        LA = 3
                        ps_y = ytp.tile([P, QG], F32, tag="yt")
                        ps_l = lpp.tile([1, QG], F32, tag="l")
                        p_tiles = {}

                        def emit_s(kt):
                            ps_s = mmp.tile([P, QG], F32, tag="mm", name="ps_s")
                            nc.tensor.matmul(
                                ps_s[:],
                                r(k_sb[h][:, kt * 128:(kt + 1) * 128]),
                                r(q_sb[h][:, qg * QG:(qg + 1) * QG]),
                                start=True,
                                stop=True,
                            )
                            p_sb = pwork.tile([P, QG], F32R, tag="p", name="p_sb")
                            nc.scalar.activation(
                                p_sb[:], ps_s[:], AF.Exp, bias=ebias[:], scale=SCALE
                            )
                            if kt >= 4 * qg:
                                s = kt - 4 * qg
                                nc.vector.tensor_mul(p_sb[:], p_sb[:], mk[:, s, :])
                            p_tiles[kt] = p_sb

                        def emit_av(kt):
                            p_sb = p_tiles.pop(kt)
                            nc.tensor.matmul(
                                ps_y[:],
                                r(v_sb[:, kt, h * 128:(h + 1) * 128]),
                                r(p_sb[:]),
                                start=(kt == 0),
                                stop=(kt == n_kt - 1),
                            )
                            nc.tensor.matmul(
                                ps_l[:],
                                r(ones_c[:]),
                                r(p_sb[:]),
                                start=(kt == 0),
                                stop=(kt == n_kt - 1),
                            )

                        for kt in range(n_kt + LA):
                            if kt < n_kt:
                                emit_s(kt)
                            if kt == 3 and pending_norm:
                                emit_norm()
                            if kt >= LA:
                                emit_av(kt - LA)
                        pending_norm.append((hg, qg, ps_y, ps_l))

                while pending_norm:
                    emit_norm()

            # ---- output projection: outT[oc, t] = wp^T-slice @ yT ----
            for hc in range(2, 4):
                nc.sync.dma_start(yt_sb[hc][:], ytr[:, hc, :])
            for oc in range(16):
                for tg in range(TG):
                    ps_o = mmp.tile([P, QG], F32, tag="mm")
                    for hc in range(4):
                        nc.tensor.matmul(
                            ps_o[:],
                            r(wp_sb[:, hc, oc * 128:(oc + 1) * 128]),
                            r(yt_sb[hc][:, tg * QG:(tg + 1) * QG]),
                            start=(hc == 0),
                            stop=(hc == 3),
                        )
                    stage = work.tile([P, QG], F32, tag="tmp")
                    nc.scalar.activation(stage[:], ps_o[:], AF.Copy)
                    nc.sync.dma_start(
                        outT[oc * 128:(oc + 1) * 128, tg * QG:(tg + 1) * QG],
                        stage[:],
                    )

    nc.finalize()
    return nc


def _host_inputs(x, freqs_cis, w_attn, w_proj):
    """Build the 8 per-core input maps."""
    x = np.asarray(x, dtype=np.float32)
    freqs_cis = np.asarray(freqs_cis, dtype=np.float32)
    w_attn = np.asarray(w_attn, dtype=np.float32)
    w_proj = np.asarray(w_proj, dtype=np.float32)

    B = x.shape[0]
    perm = np.concatenate([np.arange(0, HSIZE, 2), np.arange(1, HSIZE, 2)])

    cos = np.ascontiguousarray(freqs_cis[:, :, 0].T)  # [64, T]
    sin = np.ascontiguousarray(freqs_cis[:, :, 1].T)
    c1 = np.concatenate([cos, cos], axis=0)           # [128, T]
    c2 = np.concatenate([-sin, sin], axis=0)

    kk = np.arange(P)[:, None]
    ccol = np.arange(QG)[None, :]
    mk = np.stack(
        [(ccol >= s * 128 + kk).astype(np.float32) for s in range(4)], axis=0
    )  # [4,128,512]

    ones_col = np.ones((P, 1), np.float32)
    ones_row = np.ones((1, P), np.float32)
    swp = np.zeros((P, P), np.float32)
    for m in range(P):
        swp[(m + 64) % P, m] = 1.0

    xT = [np.ascontiguousarray(x[b].T) for b in range(B)]

    in_maps = []
    for core in range(N_CORES):
        b, g = core // 4, core % 4
        blocks = []
        for pairp in range(2):
            for off in (0, C, 2 * C):  # q, k, v origins in w_attn
                for hh in range(2):
                    hglob = 4 * g + 2 * pairp + hh
                    cols = w_attn[:, off + hglob * HSIZE: off + (hglob + 1) * HSIZE]
                    if off != 2 * C:  # permute q and k, not v
                        cols = cols[:, perm]
                    blocks.append(cols)
        wcore = np.ascontiguousarray(np.concatenate(blocks, axis=1))  # [C, 1536]
        wpcore = np.ascontiguousarray(w_proj[g * 512:(g + 1) * 512, :])
        in_maps.append(
            {
                "xT": xT[b],
                "w": wcore,
                "wp": wpcore,
                "c1": c1,
                "c2": c2,
                "mk": mk,
                "ones_col": ones_col,
                "ones_row": ones_row,
                "swp": swp,
            }
        )
    return in_maps


_LAST_RESULT = {}


def kernel(x, freqs_cis, w_attn, w_proj):
    if _TRACE:
        _install_ntff_hook()
    in_maps = _host_inputs(x, freqs_cis, w_attn, w_proj)
    nc = build_nc()
    res = run_bass_kernel_spmd(
        nc, in_maps, core_ids=list(range(N_CORES)), trace=_TRACE
    )
    _LAST_RESULT["res"] = res

    B = x.shape[0]
    out = np.zeros((B, T, C), dtype=np.float32)
    for core in range(N_CORES):
        b = core // 4
        out[b] += res.results[core]["outT"].T
    return out


# revision 16
# speedup vs baseline: 1.3063x; 1.0276x over previous
"""Causal self-attention (dense transformer block) for 8 Trainium2 NeuronCores.

Sharding: DP over batch (2) x TP over heads (4 groups of 4 heads) = 8 cores.
Per core: column-parallel QKV projection (4 heads), RoPE, causal
flash-attention (no-max-subtraction softmax with constant bias), row-parallel
output projection producing a partial [oc, t] result; host sums the 4 TP
partials per batch and transposes back.

Device matmuls run as float32r (full-rate fp32 storage, ~tf32 accuracy).

Self-contained: hardcodes shapes, builds/compiles/runs the Bass kernel via
run_bass_kernel_spmd on cores 0-7.
"""

import os
import sys
import types

sys.path.insert(0, "/opt/trn_rl_repo")

import numpy as np

import concourse.bass as bass
import concourse.mybir as mybir
import concourse.tile as tile
from concourse import bacc
from concourse.bass_utils import run_bass_kernel_spmd
from concourse.vector_clock import ScopedClock, VectorClock

F32 = mybir.dt.float32
F32R = mybir.dt.float32r
AF = mybir.ActivationFunctionType
ALU = mybir.AluOpType

P = 128
T = 2048
C = 2048
NH = 16          # total heads
HPC = 4          # heads per core
HSIZE = 128
N_CORES = 8
TG = 4           # t-groups of 512
QG = 512
EXP_BIAS = -10.0
SCALE = 1.0 / float(np.sqrt(HSIZE))

_TRACE = os.environ.get("BASS_KERNEL_TRACE", "0") == "1"


def _patch_tile_drain():
    """walrus in this toolchain allows at most one sync-wait per instruction;
    TileContext's tail drain aggregates the whole global clock onto one Drain.
    Split it: one Drain per pending proc, each with a single wait."""
    if getattr(tile.TileContext, "_drain_patched", False):
        return

    def _drain_and_barrier(self, tick_clock, wait_clock):
        nc = self.nc
        gc = tick_clock.global_clock
        n = len(gc)
        for p in range(n):
            if gc[p] > 0:
                vc = VectorClock([gc[p] if i == p else 0 for i in range(n)])
                di = nc.sync.drain()
                wait_clock.add_sem_waits(di.ins, ScopedClock({None: vc}))
        nc.all_engine_barrier()
        popped = nc._tile_sem_poison_stack.pop()
        assert popped is self._sem_poison
        nc.clear_and_free_semaphores(list(self.sems.allocated().values()))
        nc.all_engine_barrier()

    tile.TileContext._drain_and_barrier = _drain_and_barrier
    tile.TileContext._drain_patched = True


def _install_ntff_hook():
    """Wire the axon NTFF profiling hook this image leaves unwired (the agent
    image's antenv lacks axon_hooks). Only needed when tracing."""
    import antenv

    if getattr(antenv, "axon_hooks", None) is not None:
        return
    mod = types.ModuleType("antenv.axon_hooks")
    mod._hook = None
    mod.set_axon_ntff_profile_hook = lambda h: setattr(mod, "_hook", h)
    mod.get_axon_ntff_profile_hook = lambda: mod._hook
    sys.modules["antenv.axon_hooks"] = mod
    antenv.axon_hooks = mod
    if "/root/.axon_site" not in sys.path:
        sys.path.insert(0, "/root/.axon_site")
    try:
        from trn_agent_boot.trn_boot import _ntff_profile_via_ctypes

        hook = _ntff_profile_via_ctypes("/opt/axon/libaxon_pjrt.so")
        if hook is not None:
            mod.set_axon_ntff_profile_hook(hook)
        import concourse.bass_utils as bu

        bu.upload_artifacts = lambda d: d
    except Exception:
        pass


def build_nc():
    _patch_tile_drain()
    nc = bacc.Bacc(None, target_bir_lowering=False)

    xT = nc.dram_tensor("xT", [C, T], F32R, kind="ExternalInput")
    w = nc.dram_tensor("w", [C, 6 * HSIZE * 2], F32R, kind="ExternalInput")  # [C,1536]
    wp = nc.dram_tensor("wp", [HPC * HSIZE, T], F32R, kind="ExternalInput")  # [512,T]
    c1d = nc.dram_tensor("c1", [P, T], F32R, kind="ExternalInput")
    c2d = nc.dram_tensor("c2", [P, T], F32R, kind="ExternalInput")
    mkd = nc.dram_tensor("mk", [4, P, QG], F32R, kind="ExternalInput")
    onesd = nc.dram_tensor("ones_col", [P, 1], F32R, kind="ExternalInput")
    onesrd = nc.dram_tensor("ones_row", [1, P], F32R, kind="ExternalInput")
    swpd = nc.dram_tensor("swp", [P, P], F32R, kind="ExternalInput")
    outT = nc.dram_tensor("outT", [T, T], F32, kind="ExternalOutput")  # [oc, t]

    xTr = xT.rearrange("(cc p) t -> p cc t", p=P)      # [128,16,2048]
    wr = w.rearrange("(cc p) j -> p cc j", p=P)        # [128,16,1536]
    wpr = wp.rearrange("(hc p) t -> p hc t", p=P)      # [128,4,2048]
    mkr = mkd.rearrange("s p q -> p s q")              # [128,4,512]

    def r(ap):
        return ap

    with tile.TileContext(nc) as tc, nc.allow_low_precision(
        reason="f32r storage is the intended reduced-precision matmul format"
    ):
        with (
            tc.tile_pool(name="const", bufs=1) as constp,
            tc.tile_pool(name="wpool", bufs=1) as wpool,
            tc.tile_pool(name="xtp", bufs=5) as xtp,
            tc.tile_pool(name="qk", bufs=1) as qkres,
            tc.tile_pool(name="vres", bufs=1) as vresp,
            tc.tile_pool(name="work", bufs=8) as work,
            tc.tile_pool(name="pwork", bufs=10) as pwork,
            tc.tile_pool(name="rp", bufs=4) as rpool,
            tc.tile_pool(name="mm", bufs=4, space="PSUM") as mmp,
            tc.tile_pool(name="yt", bufs=2, space="PSUM") as ytp,
            tc.tile_pool(name="lp", bufs=2, space="PSUM") as lpp,
            tc.tile_pool(name="dram", bufs=1, space="DRAM") as dramp,
        ):
            c1 = constp.tile([P, T], F32R, tag="c1")
            c2 = constp.tile([P, T], F32R, tag="c2")
            mk = constp.tile([P, 4, QG], F32R, tag="mk")
            ones_c = constp.tile([P, 1], F32R, tag="onc")
            ones_r = constp.tile([1, P], F32R, tag="onr")
            swp = constp.tile([P, P], F32R, tag="swp")
            nc.sync.dma_start(c1[:], c1d[:])
            nc.sync.dma_start(c2[:], c2d[:])
            nc.sync.dma_start(mk[:], mkr)
            nc.sync.dma_start(ones_c[:], onesd[:])
            ebias = constp.tile([P, 1], F32, tag="ebias")
            nc.gpsimd.memset(ebias[:], EXP_BIAS)
            nc.sync.dma_start(ones_r[:], onesrd[:])
            nc.sync.dma_start(swp[:], swpd[:])

            yt_dram = dramp.tile([HPC * HSIZE, T], F32R)  # [512, 2048] spill
            ytr = yt_dram.rearrange("(hc p) t -> p hc t", p=P)
            yt_sb = []

            for pair in range(2):
                w_sb = wpool.tile([P, 16, 768], F32R, tag="w")
                nc.sync.dma_start(w_sb[:], wr[:, :, pair * 768:(pair + 1) * 768])

                q_sb = [qkres.tile([P, T], F32R, tag=f"q{h}", name=f"q{h}") for h in range(2)]
                k_sb = [qkres.tile([P, T], F32R, tag=f"k{h}", name=f"k{h}") for h in range(2)]
                v_sb = vresp.tile([P, 16, 256], F32R, tag="v")

                # ---- QKV projection for this pair ----
                for tg in range(TG):
                    xts = []
                    for ch in range(4):
                        xt = xtp.tile([P, 4, QG], F32R, tag="xt")
                        nc.sync.dma_start(
                            xt[:],
                            xTr[:, ch * 4:(ch + 1) * 4, tg * QG:(tg + 1) * QG],
                        )
                        xts.append(xt)

                    # q/k: 4 j-tiles (q_h0, q_h1, k_h0, k_h1), N=512,
                    # in two passes of 2 concurrent psums to keep mm-pool slack
                    def rope(j, psum):
                        dst = (q_sb if j < 2 else k_sb)[j % 2]
                        dsl = dst[:, tg * QG:(tg + 1) * QG]
                        qraw = work.tile([P, QG], F32R, tag="tmp", name="qraw")
                        nc.scalar.activation(qraw[:], psum[:], AF.Copy)
                        ps_sw = mmp.tile([P, QG], F32, tag="mm", name="ps_sw")
                        nc.tensor.matmul(
                            ps_sw[:], swp[:], qraw[:], start=True, stop=True
                        )
                        t2 = work.tile([P, QG], F32R, tag="tmp", name="t2")
                        c1s = c1[:, tg * QG:(tg + 1) * QG]
                        c2s = c2[:, tg * QG:(tg + 1) * QG]
                        nc.vector.tensor_mul(dsl, qraw[:], c1s)
                        nc.vector.tensor_mul(t2[:], ps_sw[:], c2s)
                        nc.vector.tensor_add(dsl, dsl, t2[:])

                    for jp in range(2):
                        psq = [mmp.tile([P, QG], F32, tag="mm", name=f"psq{j}")
                               for j in range(2)]
                        for cc in range(16):
                            xt = xts[cc // 4][:, cc % 4, :]
                            for j in range(2):
                                nc.tensor.matmul(
                                    psq[j][:],
                                    r(w_sb[:, cc, (jp * 2 + j) * 128:(jp * 2 + j + 1) * 128]),
                                    r(xt),
                                    start=(cc == 0),
                                    stop=(cc == 15),
                                )
                        rope(jp * 2 + 0, psq[0])
                        rope(jp * 2 + 1, psq[1])
                    # v: 4 t-tiles in this tg, N=256 (both heads' v); yt pool is idle here
                    for tt in range(4):
                        psv = ytp.tile([P, 256], F32, tag="yt", name="psv")
                        for cc in range(16):
                            nc.tensor.matmul(
                                psv[:],
                                r(xts[cc // 4][:, cc % 4, tt * 128:(tt + 1) * 128]),
                                r(w_sb[:, cc, 512:768]),
                                start=(cc == 0),
                                stop=(cc == 15),
                            )
                        nc.vector.tensor_copy(v_sb[:, tg * 4 + tt, :], psv[:])

                # ---- attention for the pair's 2 heads ----
                if pair == 1:
                    # prefetch proj weights + pair0's yT during pair1 attention
                    wp_sb = wpool.tile([P, 4, T], F32R, tag="w", name="wp_sb")
                    for ocq in range(4):
                        nc.sync.dma_start(
                            wp_sb[:, :, ocq * 512:(ocq + 1) * 512],
                            wpr[:, :, ocq * 512:(ocq + 1) * 512],
                        )
                    yt_sb.extend(
                        xtp.tile([P, T], F32R, tag="xt", name=f"yt{i}")
                        for i in range(4)
                    )
                    for hc in range(2):
                        nc.sync.dma_start(yt_sb[hc][:], ytr[:, hc, :])

                pending_norm = []

                def emit_norm():
                    hg_, qg_, ps_y_, ps_l_ = pending_norm.pop(0)
                    r_f32 = rpool.tile([1, QG], F32, tag="rf", name="r_f32")
                    nc.vector.reciprocal_approx_fast(r_f32[:], ps_l_[:])
                    r128 = pwork.tile([P, QG], F32, tag="p", name="r128")
                    nc.gpsimd.partition_broadcast(r128[:], r_f32[0:1, :])
                    if hg_ >= 2:
                        # pair1: normalize straight into the resident proj input
                        nc.vector.tensor_mul(
                            yt_sb[hg_][:, qg_ * QG:(qg_ + 1) * QG], ps_y_[:], r128[:]
                        )
                    else:
                        stage = work.tile([P, QG], F32R, tag="tmp", name="stage")
                        nc.vector.tensor_mul(stage[:], ps_y_[:], r128[:])
                        nc.sync.dma_start(
                            yt_dram[hg_ * 128:(hg_ + 1) * 128,
                                    qg_ * QG:(qg_ + 1) * QG],
                            stage[:],
                        )

                for h in range(2):
                    hg = pair * 2 + h
                    for qg in range(TG):
                        n_kt = 4 * qg + 4
                        LA = 3
                        ps_y = ytp.tile([P, QG], F32, tag="yt")
                        ps_l = lpp.tile([1, QG], F32, tag="l")
                        p_tiles = {}

                        def emit_s(kt):
                            ps_s = mmp.tile([P, QG], F32, tag="mm", name="ps_s")
                            nc.tensor.matmul(
                                ps_s[:],
                                r(k_sb[h][:, kt * 128:(kt + 1) * 128]),
                                r(q_sb[h][:, qg * QG:(qg + 1) * QG]),
                                start=True,
                                stop=True,
                            )
                            p_sb = pwork.tile([P, QG], F32R, tag="p", name="p_sb")
                            nc.scalar.activation(
                                p_sb[:], ps_s[:], AF.Exp, bias=ebias[:], scale=SCALE
                            )
                            if kt >= 4 * qg:
                                s = kt - 4 * qg
                                nc.vector.tensor_mul(p_sb[:], p_sb[:], mk[:, s, :])
                            p_tiles[kt] = p_sb

                        def emit_av(kt):
                            p_sb = p_tiles.pop(kt)
                            nc.tensor.matmul(
                                ps_y[:],
                                r(v_sb[:, kt, h * 128:(h + 1) * 128]),
                                r(p_sb[:]),
                                start=(kt == 0),
                                stop=(kt == n_kt - 1),
                            )
                            nc.tensor.matmul(
                                ps_l[:],
                                r(ones_c[:]),
                                r(p_sb[:]),
                                start=(kt == 0),
                                stop=(kt == n_kt - 1),
                            )

                        for kt in range(n_kt + LA):
                            if kt < n_kt:
                                emit_s(kt)
                            if kt == 3 and pending_norm:
                                emit_norm()
                            if kt >= LA:
                                emit_av(kt - LA)
                        pending_norm.append((hg, qg, ps_y, ps_l))

                while pending_norm:
                    emit_norm()

            # ---- output projection: outT[oc, t] = wp^T-slice @ yT ----
            for oc in range(16):
                for tg in range(TG):
                    ps_o = mmp.tile([P, QG], F32, tag="mm")
                    for hc in range(4):
                        nc.tensor.matmul(
                            ps_o[:],
                            r(wp_sb[:, hc, oc * 128:(oc + 1) * 128]),
                            r(yt_sb[hc][:, tg * QG:(tg + 1) * QG]),
                            start=(hc == 0),
                            stop=(hc == 3),
                        )
                    stage = work.tile([P, QG], F32, tag="tmp")
                    nc.scalar.activation(stage[:], ps_o[:], AF.Copy)
                    nc.sync.dma_start(
                        outT[oc * 128:(oc + 1) * 128, tg * QG:(tg + 1) * QG],
                        stage[:],
                    )

    nc.finalize()
    return nc


def _host_inputs(x, freqs_cis, w_attn, w_proj):
    """Build the 8 per-core input maps."""
    x = np.asarray(x, dtype=np.float32)
    freqs_cis = np.asarray(freqs_cis, dtype=np.float32)
    w_attn = np.asarray(w_attn, dtype=np.float32)
    w_proj = np.asarray(w_proj, dtype=np.float32)

    B = x.shape[0]
    perm = np.concatenate([np.arange(0, HSIZE, 2), np.arange(1, HSIZE, 2)])

    cos = np.ascontiguousarray(freqs_cis[:, :, 0].T)  # [64, T]
    sin = np.ascontiguousarray(freqs_cis[:, :, 1].T)
    c1 = np.concatenate([cos, cos], axis=0)           # [128, T]
    c2 = np.concatenate([-sin, sin], axis=0)

    kk = np.arange(P)[:, None]
    ccol = np.arange(QG)[None, :]
    mk = np.stack(
        [(ccol >= s * 128 + kk).astype(np.float32) for s in range(4)], axis=0
    )  # [4,128,512]

    ones_col = np.ones((P, 1), np.float32)
    ones_row = np.ones((1, P), np.float32)
    swp = np.zeros((P, P), np.float32)
    for m in range(P):
        swp[(m + 64) % P, m] = 1.0

    xT = [np.ascontiguousarray(x[b].T) for b in range(B)]

    in_maps = []
    for core in range(N_CORES):
        b, g = core // 4, core % 4
        blocks = []
        for pairp in range(2):
            for off in (0, C, 2 * C):  # q, k, v origins in w_attn
                for hh in range(2):
                    hglob = 4 * g + 2 * pairp + hh
                    cols = w_attn[:, off + hglob * HSIZE: off + (hglob + 1) * HSIZE]
                    if off != 2 * C:  # permute q and k, not v
                        cols = cols[:, perm]
                    blocks.append(cols)
        wcore = np.ascontiguousarray(np.concatenate(blocks, axis=1))  # [C, 1536]
        wpcore = np.ascontiguousarray(w_proj[g * 512:(g + 1) * 512, :])
        in_maps.append(
            {
                "xT": xT[b],
                "w": wcore,
                "wp": wpcore,
                "c1": c1,
                "c2": c2,
                "mk": mk,
                "ones_col": ones_col,
                "ones_row": ones_row,
                "swp": swp,
            }
        )
    return in_maps


_LAST_RESULT = {}


def kernel(x, freqs_cis, w_attn, w_proj):
    if _TRACE:
        _install_ntff_hook()
    in_maps = _host_inputs(x, freqs_cis, w_attn, w_proj)
    nc = build_nc()
    res = run_bass_kernel_spmd(
        nc, in_maps, core_ids=list(range(N_CORES)), trace=_TRACE
    )
    _LAST_RESULT["res"] = res

    B = x.shape[0]
    out = np.zeros((B, T, C), dtype=np.float32)
    for core in range(N_CORES):
        b = core // 4
        out[b] += res.results[core]["outT"].T
    return out
